# revision 79
# baseline (speedup 1.0000x reference)
"""GQA attention block (B=1, T=2048, HID=2048, NQ=16, NKV=8, D=128) on 8 TRN2
NeuronCores.

Sharding: tensor-parallel over heads. Core c owns q-heads {2c, 2c+1} and
kv-head c. Each core computes, from the full x:
  Q^T/K^T shards (d on partitions) and V in [token, d] layout (via a
  transposed projection, so no PE transposes)  ->  per-head RMSNorm + RoPE
  ->  causal softmax attention (no max-subtraction; scores are O(5) for
  RMS-normed q/k; diagonal blocks trimmed to the causal width)  ->  partial
  o_proj with Wo row-shard, written out fp16.
The 8 partial [T, HID] outputs are summed on the host (the row-parallel
"unshard" step).

All matmul operands are fp16 (full PE rate at any free-dim size; rel err
~5e-4, far under the 2e-2 gate -- fp8 would blow the budget); PSUM
accumulation stays fp32. Softmax denominators are accumulated on the vector
engine (fp16, 2x mode) and partition-all-reduced on the otherwise-idle Pool
engine, keeping the PE free of reduction matmuls; RMS statistics use the
same trick. Scheduling is a single tile scope, software-pipelined so no
phase barrier drains the machine:
  - iteration tr: q projections -> attention score phase for qr=tr-1 ->
    deferred RMS/RoPE chains (their Act Sqrt would otherwise head-of-line
    block the exps; a dummy exp then prefetches the Act table flip) -> k/v
    projections -> attention context phase,
  - o_proj is emitted as single-PSUM-tile units woven into the exp-starved
    score stretches (Act exp throughput is the binding resource there),
    with PSUM->SBUF fp16 copies split between DVE and Act,
  - the last query-range's attention weaves qr=2's context phase and v(3)
    in as extra PE filler, and normalizes per half so o_proj(3) starts
    progressively.
DMA: transfers are FIFO in desc-gen order, so startup-critical loads ride
the Act HWDGE queue in exact need-order while x0 drips on SP; out-writes
are batched per 128-row block (HWDGE desc-gen is a serial 625ns/op
resource).
"""

import sys
from collections import deque

sys.path.insert(0, "/opt/trn_rl_repo")

import numpy as np

import concourse.bass as bass  # noqa: F401  (bass must import before tile)
import concourse.bass_isa as bass_isa
import concourse.mybir as mybir
import concourse.tile as tile
from concourse import bacc
from concourse.bass_utils import run_bass_kernel_spmd

N_CORES = 8
T = 2048
HID = 2048
NQ, NKV, D = 16, 8, 128
HQ = NQ // N_CORES  # q heads per core = 2
EPS = 1e-6
SCALE = D**-0.5

P = 128
H = D // 2           # rope half
NK = HID // P        # 16 k-chunks for projections
NTR = T // 512       # 4 T-ranges of 512
NTT = T // P         # 16 T-tiles of 128

F32 = mybir.dt.float32
F16 = mybir.dt.float16
ACT_EXP = mybir.ActivationFunctionType.Exp
ACT_SQUARE = mybir.ActivationFunctionType.Square
ACT_SQRT = mybir.ActivationFunctionType.Sqrt
RED_ADD = bass_isa.ReduceOp.add


def build_nc():
    nc = bacc.Bacc("TRN2", target_bir_lowering=False, debug=False,
                   num_devices=N_CORES)

    # ---- DRAM tensors (names = in_map keys); all pre-arranged on host ----
    xt = nc.dram_tensor("xt", [P, NK, T], F16, kind="ExternalInput")
    wq0 = nc.dram_tensor("wq0", [P, NK, D], F16, kind="ExternalInput")
    wq1 = nc.dram_tensor("wq1", [P, NK, D], F16, kind="ExternalInput")
    wkv = nc.dram_tensor("wkv", [P, NK, 2 * D], F16, kind="ExternalInput")
    wo = nc.dram_tensor("wo", [P, HQ, HID], F16, kind="ExternalInput")
    cosT = nc.dram_tensor("cosT", [D, T], F16, kind="ExternalInput")
    sinT = nc.dram_tensor("sinT", [D, T], F16, kind="ExternalInput")
    qw = nc.dram_tensor("qw", [D, 1], F32, kind="ExternalInput")
    kw = nc.dram_tensor("kw", [D, 1], F32, kind="ExternalInput")
    masks = nc.dram_tensor("masks", [P, 4, 512], F16, kind="ExternalInput")
    out = nc.dram_tensor("out", [T, HID], F16, kind="ExternalOutput")

    with tile.TileContext(nc) as tc:
        with (
            tc.tile_pool(name="cst", bufs=1) as cst,
            tc.tile_pool(name="fin", bufs=1) as fin,
            tc.tile_pool(name="xtp", bufs=2) as xtp,
            tc.tile_pool(name="rawp", bufs=2) as rawp,
            tc.tile_pool(name="sqp", bufs=3) as sqp,
            tc.tile_pool(name="rmsp", bufs=2) as rmsp,
            tc.tile_pool(name="ropep", bufs=3) as ropep,
            tc.tile_pool(name="atp", bufs=36) as atp,
            tc.tile_pool(name="accp", bufs=3) as accp,
            tc.tile_pool(name="dnp", bufs=2) as dnp,
            tc.tile_pool(name="otp", bufs=3) as otp,
            tc.tile_pool(name="psA", bufs=2, space="PSUM") as psA,
            tc.tile_pool(name="psS", bufs=3, space="PSUM") as psS,
            tc.tile_pool(name="psO", bufs=3, space="PSUM") as psO,
        ):
            # ---------- constants / weights resident in SBUF ----------
            wq_sb = [cst.tile([P, NK, D], F16, name=f"wq{h}_sb")
                     for h in range(HQ)]
            wkv_sb = cst.tile([P, NK, 2 * D], F16)
            wo_sb = cst.tile([P, HQ, HID], F16)
            cos_sb = cst.tile([P, T], F16)
            sin_sb = cst.tile([P, T], F16)   # pre-rolled, first half negated
            masks_sb = cst.tile([P, 4, 512], F16)
            qw_sb = cst.tile([P, 1], F32)
            kw_sb = cst.tile([P, 1], F32)
            eps_sb = cst.tile([P, 1], F32)
            nc.gpsimd.memset(eps_sb[:], EPS)
            dmy = cst.tile([1, 1], F32)
            nc.gpsimd.memset(dmy[:], 0.0)
            dmy_o = cst.tile([1, 1], F16)

            def preload_exp():
                # A do-nothing exp: forces the Act function-table switch
                # (1.3us) to happen NOW, while Act is idle, instead of right
                # in front of the first real exp of the attention phase.
                nc.scalar.activation(dmy_o[:], dmy[:], ACT_EXP)

            # DMA transfers run FIFO in desc-gen order, and desc-gen follows
            # per-queue program order -- so everything startup-critical rides
            # the Act HWDGE queue in exact need-order (weights, then x1 and
            # trig tables interleaved), x0/x2/x3/out ride SP, and wo (needed
            # only by o_proj at ~40us) is emitted inside iteration 1 on the
            # Pool SWDGE queue so it cannot jump the early queue.
            # x0's first drip goes before the weights so its desc-gen (and
            # transfer) is first in the FIFO; the rest interleave by need.
            xch01 = []
            for tr01 in range(2):
                xch_t = xtp.tile([P, NK, 512], F16, name="xch")
                t0 = slice(tr01 * 512, (tr01 + 1) * 512)
                if tr01 == 0:
                    # drip x0 in 4 chunks so the first projection group can
                    # chew k-chunks while the rest stream in
                    for ks in (slice(0, 2), slice(2, 5), slice(5, 10),
                               slice(10, 16)):
                        nc.sync.dma_start(xch_t[:, ks, :], xt[:, ks, t0])
                    nc.scalar.dma_start(wq_sb[0][:], wq0[:])
                    nc.scalar.dma_start(qw_sb[:], qw[:])
                    nc.scalar.dma_start(kw_sb[:], kw[:])
                    nc.scalar.dma_start(wq_sb[1][:], wq1[:])
                    nc.scalar.dma_start(wkv_sb[:], wkv[:])
                else:
                    nc.scalar.dma_start(xch_t[:, 0:8, :], xt[:, 0:8, t0])
                    nc.scalar.dma_start(xch_t[:, 8:, :], xt[:, 8:, t0])
                    nc.scalar.dma_start(cos_sb[:], cosT[:])
                    nc.scalar.dma_start(sin_sb[:], sinT[:])
                    nc.scalar.dma_start(masks_sb[:], masks[:])
                    nc.scalar.dma_start(wo_sb[:], wo[:])
                xch01.append(xch_t)

            # final (post RMS+RoPE) activations, fp16
            qT = [fin.tile([P, T], F16, name=f"qT{h}") for h in range(HQ)]
            kT = fin.tile([P, T], F16)
            vnat = fin.tile([P, NTT, D], F16)
            ctxT = [
                [fin.tile([P, 512], F16, name=f"ctxT{h}_{qr}")
                 for qr in range(NTR)]
                for h in range(HQ)
            ]

            COLS = {
                "q0": (qT[0], (lambda: wq_sb[0]), 0, qw_sb),
                "q1": (qT[1], (lambda: wq_sb[1]), 0, qw_sb),
                "k": (kT, (lambda: wkv_sb), 0, kw_sb),
            }

            def emit_proj_mm(tr, xch, name):
                """Projection matmul group + psum evacuation (Act Copy +
                Square only -- both table-neutral, so they never delay the
                attention exps that follow on the Act queue)."""
                _, w_fn, off, _ = COLS[name]
                w_sb = w_fn()
                ps = psA.tile([P, 512], F32, name="psA_t")
                for k in range(NK):
                    nc.tensor.matmul(
                        ps[:], w_sb[:, k, off:off + D], xch[:, k, :],
                        start=(k == 0), stop=(k == NK - 1),
                    )
                sq = sqp.tile([P, 512], F16, name="sq")
                nc.scalar.activation(sq[:], ps[:], ACT_SQUARE)
                raw = rawp.tile([P, 512], F32, name=f"raw_{name}")
                nc.scalar.copy(raw[:], ps[:])
                return raw, sq

            def emit_bchain(tr, name, raw, sq):
                """Deferred RMS+RoPE chain (contains the Act Sqrt, so it is
                emitted AFTER the attention score phase: its table flip then
                lands in Act slack, not in front of the exps)."""
                ts = slice(tr * 512, (tr + 1) * 512)
                dst, _, _, nrm_w = COLS[name]
                ssum_b = rmsp.tile([P, 512], F32, name="ssum_b")
                nc.gpsimd.partition_all_reduce(ssum_b[:], sq[:],
                                               channels=P,
                                               reduce_op=RED_ADD)
                rstd_b = rmsp.tile([P, 512], F32, name="rstd_b")
                nc.scalar.activation(rstd_b[:], ssum_b[:], ACT_SQRT,
                                     scale=1.0 / D, bias=eps_sb[:])
                rinv_b = rmsp.tile([P, 512], F32, name="rinv_b")
                nc.vector.reciprocal_approx_fast(rinv_b[:], rstd_b[:])
                nq = ropep.tile([P, 512], F16, name="nq")
                nc.vector.scalar_tensor_tensor(
                    nq[:], raw[:], nrm_w[:], rinv_b[:],
                    mybir.AluOpType.mult, mybir.AluOpType.mult,
                )
                pc = ropep.tile([P, 512], F16, name="pc")
                nc.vector.tensor_mul(pc[:], nq[:], cos_sb[:, ts])
                psn = ropep.tile([P, 512], F16, name="psn")
                nc.vector.tensor_mul(psn[0:H, :], nq[H:D, :],
                                     sin_sb[H:D, ts])
                nc.vector.tensor_mul(psn[H:D, :], nq[0:H, :],
                                     sin_sb[0:H, ts])
                nc.vector.tensor_add(dst[:, ts], pc[:], psn[:])

            def emit_proj_v(tr, xch):
                # --- v : out [128 t, 128 d] per t-tile, no transpose
                vps = psA.tile([P, 512], F32, name="psA_t")
                for j in range(4):
                    for k in range(NK):
                        nc.tensor.matmul(
                            vps[:, j * P:(j + 1) * P],
                            xch[:, k, j * P:(j + 1) * P],
                            wkv_sb[:, k, D:2 * D],
                            start=(k == 0), stop=(k == NK - 1),
                        )
                nc.vector.tensor_copy(vnat[:, 4 * tr:4 * tr + 4, :],
                                      vps[:])

            def emit_scores(h, qr, mids=(), filler=None, filler_from=0):
                """Score matmuls + exp + causal mask + fp16 denominator
                accumulation for one (head, query-range). Returns (ats,
                qoffs, acc). mids: (st, callback) pairs weaving in big PE
                blocks; filler: a deque of small PE units (single o_proj
                tiles) popped every other st to soak up exp latency."""
                n_st = 4 * (qr + 1)
                ats, qoffs = [], []
                acc = accp.tile([P, 512], F16, name="acc")
                for st in range(n_st):
                    for st_m, cb in mids:
                        if st == st_m:
                            cb()
                    if filler and st % 2 == 1 and st >= filler_from:
                        filler.popleft()()
                    j = st - 4 * qr
                    qoff = max(0, 128 * j) if j >= 0 else 0
                    s_ps = psS.tile([P, 512], F32, name="s_ps")
                    nc.tensor.matmul(
                        s_ps[:, qoff:], kT[:, st * P:(st + 1) * P],
                        qT[h][:, qr * 512 + qoff:(qr + 1) * 512],
                        start=True, stop=True,
                    )
                    at = atp.tile([P, 512], F16, name="at")
                    nc.scalar.activation(at[:, qoff:], s_ps[:, qoff:],
                                         ACT_EXP, scale=SCALE)
                    if j >= 0:
                        nc.vector.tensor_mul(at[:, qoff:], at[:, qoff:],
                                             masks_sb[:, j, qoff:])
                    # accumulate into acc on DVE (fp16 2x mode)
                    if st == 1:
                        if qoffs[0] == 0 and qoff == 0:
                            nc.vector.tensor_add(acc[:], ats[0][:], at[:])
                        else:  # qr == 0: at0 full, at1 starts at 128
                            nc.vector.tensor_copy(acc[:, :qoff],
                                                  ats[0][:, :qoff])
                            nc.vector.tensor_add(acc[:, qoff:],
                                                 ats[0][:, qoff:],
                                                 at[:, qoff:])
                    elif st >= 2:
                        nc.vector.tensor_add(acc[:, qoff:], acc[:, qoff:],
                                             at[:, qoff:])
                    ats.append(at)
                    qoffs.append(qoff)
                return ats, qoffs, acc

            def emit_ctx(h, qr, ats, qoffs, acc, filler=None):
                """Context matmuls + denominator all-reduce + normalize.
                For the final query-range the normalize runs per 128-query
                quarter so o_proj(3) t-tiles can start progressively."""
                n_st = 4 * (qr + 1)
                ctx_ps = psO.tile([P, 512], F32, name="psO_t")
                for st in range(n_st):
                    if filler and st % 2 == 1:
                        filler.popleft()()
                    qoff = qoffs[st]
                    nc.tensor.matmul(
                        ctx_ps[:, qoff:], vnat[:, st, :], ats[st][:, qoff:],
                        start=(st == 0), stop=(st == n_st - 1),
                    )
                denom_b = dnp.tile([P, 512], F32, name="denom_b")
                rb = dnp.tile([P, 512], F32, name="rb")
                nc.gpsimd.partition_all_reduce(denom_b[:], acc[:],
                                               channels=P,
                                               reduce_op=RED_ADD)
                halves = (
                    [slice(0, 256), slice(256, 512)]
                    if qr == NTR - 1 else [slice(0, 512)]
                )
                for qs4 in halves:
                    nc.vector.reciprocal_approx_fast(rb[:, qs4],
                                                     denom_b[:, qs4])
                    nc.vector.tensor_mul(ctxT[h][qr][:, qs4],
                                         ctx_ps[:, qs4], rb[:, qs4])

            oproj_n = [0]

            def emit_oproj(qr, tts, act_every=4, split_dma=False):
                """o_proj for t-tiles tts of query-range qr. PSUM -> fp16
                SBUF copies (GPSIMD can't read PSUM on hw) split between DVE
                and Act (every act_every-th on Act); the 4 n-range tiles of
                one t-tile stage into one [P, HID] buffer so a single
                batched DMA covers the whole row block (HWDGE desc-gen is a
                serial 625ns/op resource -- 16 DMAs, not 64). split_dma
                switches the last tile back to per-n-range DMAs so the
                drain tail is fine-grained."""
                for tt in tts:
                    off = (tt % 4) * P
                    ot = otp.tile([P, HID], F16, name="ot")
                    last = split_dma and tt == tts[-1]
                    for nr in range(NTR):
                        ns = slice(nr * 512, (nr + 1) * 512)
                        ps = psO.tile([P, 512], F32, name="psO_t")
                        for h in range(HQ):
                            nc.tensor.matmul(
                                ps[:], ctxT[h][qr][:, off:off + P],
                                wo_sb[:, h, ns],
                                start=(h == 0), stop=(h == HQ - 1),
                            )
                        use_act = oproj_n[0] % act_every == act_every - 1
                        if use_act:
                            nc.scalar.copy(ot[:, ns], ps[:])
                        else:
                            nc.vector.tensor_copy(ot[:, ns], ps[:])
                        oproj_n[0] += 1
                        if last:
                            nc.sync.dma_start(
                                out[tt * P:(tt + 1) * P, ns], ot[:, ns])
                    if not last:
                        nc.sync.dma_start(out[tt * P:(tt + 1) * P, :], ot[:])

            def oproj_units(qr, tts, act_every=4):
                """Per-(t-tile, n-range) o_proj emission units for fine
                weaving into exp-starved stretches."""
                units, ots = [], {}
                for tt in tts:
                    for nr in range(NTR):
                        def u(tt=tt, nr=nr):
                            off = (tt % 4) * P
                            if nr == 0:
                                ots[tt] = otp.tile([P, HID], F16, name="ot")
                            ot = ots[tt]
                            ns = slice(nr * 512, (nr + 1) * 512)
                            ps = psO.tile([P, 512], F32, name="psO_t")
                            for h in range(HQ):
                                nc.tensor.matmul(
                                    ps[:], ctxT[h][qr][:, off:off + P],
                                    wo_sb[:, h, ns],
                                    start=(h == 0), stop=(h == HQ - 1),
                                )
                            use_act = (oproj_n[0] % act_every
                                       == act_every - 1)
                            if use_act:
                                nc.scalar.copy(ot[:, ns], ps[:])
                            else:
                                nc.vector.tensor_copy(ot[:, ns], ps[:])
                            oproj_n[0] += 1
                            if nr == NTR - 1:
                                nc.sync.dma_start(
                                    out[tt * P:(tt + 1) * P, :], ot[:])
                        units.append(u)
                return units

            def emit_attn_a(qr, filler, extra=()):
                """Score phase for both heads of query-range qr; filler
                units (o_proj tiles of qr-1) pop every other st."""
                ex = list(extra)
                if ex:  # last-qr: qr-2's ctx blocks lead (they are the
                    # early filler and the units depend on their output)
                    m0 = [(1, ex[0]), (4, ex[1])]
                    m1 = [(2, ex[2])]
                    a0 = emit_scores(0, qr, mids=m0, filler=filler,
                                     filler_from=6)
                    a1 = emit_scores(1, qr, mids=m1, filler=filler)
                else:
                    a0 = emit_scores(0, qr, filler=filler)
                    a1 = emit_scores(1, qr, filler=filler)
                return a0, a1

            def emit_attn_b(qr, a0, a1, filler=None):
                """Context phase for both heads of query-range qr; drains
                any remaining filler units at the end."""
                emit_ctx(0, qr, *a0, filler=filler)
                emit_ctx(1, qr, *a1, filler=filler)
                while filler:
                    filler.popleft()()

            # ==================== main pipelined loop ====================
            for tr in range(NTR):
                ts = slice(tr * 512, (tr + 1) * 512)
                if tr < 2:
                    xch = xch01[tr]
                else:
                    xch = xtp.tile([P, NK, 512], F16, name="xch")
                    for kg in range(2):
                        ks = slice(kg * 8, (kg + 1) * 8)
                        nc.sync.dma_start(xch[:, ks, :], xt[:, ks, ts])
                rq0 = emit_proj_mm(tr, xch, "q0")
                rq1 = emit_proj_mm(tr, xch, "q1")
                filler = None
                if tr >= 2:
                    filler = deque(oproj_units(
                        tr - 2, list(range(4 * (tr - 2), 4 * (tr - 1)))))
                aa = emit_attn_a(tr - 1, filler) if tr >= 1 else None
                emit_bchain(tr, "q0", *rq0)
                emit_bchain(tr, "q1", *rq1)
                rk = emit_proj_mm(tr, xch, "k")
                emit_bchain(tr, "k", *rk)
                preload_exp()
                if tr < NTR - 1:
                    emit_proj_v(tr, xch)
                else:
                    xch_last = xch  # v(3) woven into attn(3)'s score phase
                if aa is not None:
                    if tr < NTR - 1:
                        emit_attn_b(tr - 1, *aa, filler=filler)
                    else:
                        # last iteration: qr=2's ctx phase is woven into
                        # attn(3)'s exp-starved score phase instead
                        aa_prev = aa
                        filler_prev = filler
            # drain any oproj(1) units not yet emitted, then the final
            # region: attn(3) with ctx(2)/v(3) as leading filler and
            # oproj(2) units woven through, then oproj(3).
            while filler_prev:
                filler_prev.popleft()()
            filler = deque(oproj_units(
                NTR - 2, list(range(4 * (NTR - 2), 4 * (NTR - 1)))))
            a2_0, a2_1 = aa_prev
            aa = emit_attn_a(
                NTR - 1, filler,
                extra=(lambda: emit_ctx(0, NTR - 2, *a2_0),
                       lambda: emit_ctx(1, NTR - 2, *a2_1),
                       lambda: emit_proj_v(NTR - 1, xch_last)),
            )
            emit_attn_b(NTR - 1, *aa, filler=filler)
            emit_oproj(NTR - 1, list(range(4 * (NTR - 1), 4 * NTR)),
                       act_every=2, split_dma=True)

    nc.compile()
    return nc


_NC_CACHE = None


def get_nc():
    global _NC_CACHE
    if _NC_CACHE is None:
        _NC_CACHE = build_nc()
    return _NC_CACHE


def make_in_maps(x, cos, sin, Wq, Wk, Wv, Wo, q_norm_w, k_norm_w):
    x = np.asarray(x, dtype=np.float32).reshape(T, HID)
    # xt: [P, NK, T] fp16, HID index = k*P + p
    xt = np.ascontiguousarray(
        x.T.reshape(NK, P, T).transpose(1, 0, 2).astype(np.float16)
    )
    cosT = np.ascontiguousarray(np.asarray(cos, np.float32).T)
    # sin, transposed, first half negated, then rolled by 64 partitions:
    # psn = rot_half-mul uses sin_sb[H:] for dst[:H] (needs -sin[:H]) and
    # sin_sb[:H] for dst[H:] (needs +sin[H:]); single add then applies RoPE.
    sin_t = np.asarray(sin, np.float32).T.copy()
    sin_t[:H] *= -1.0
    sinT = np.ascontiguousarray(np.roll(sin_t, H, axis=0))
    qwv = np.ascontiguousarray(np.asarray(q_norm_w, np.float32).reshape(D, 1))
    kwv = np.ascontiguousarray(np.asarray(k_norm_w, np.float32).reshape(D, 1))
    si = np.arange(P)[:, None, None]
    jj = np.arange(4)[None, :, None]
    qi = np.arange(512)[None, None, :]
    masks = (si + P * jj <= qi).astype(np.float16)
    Wq = np.asarray(Wq, np.float32)
    Wk = np.asarray(Wk, np.float32)
    Wv = np.asarray(Wv, np.float32)
    Wo = np.asarray(Wo, np.float32)
    in_maps = []
    for c in range(N_CORES):
        wq_c = Wq[:, c * HQ * D:(c + 1) * HQ * D]      # [HID, 256]
        wk_c = Wk[:, c * D:(c + 1) * D]                # [HID, 128]
        wv_c = Wv[:, c * D:(c + 1) * D]                # [HID, 128]
        wkv_c = np.concatenate([wk_c, wv_c], axis=1)   # [HID, 256]
        wo_c = Wo[c * HQ * D:(c + 1) * HQ * D, :]      # [256, HID]
        in_maps.append({
            "xt": xt,
            "wq0": np.ascontiguousarray(
                wq_c[:, 0:D].reshape(NK, P, D).transpose(1, 0, 2)
            ).astype(np.float16),
            "wq1": np.ascontiguousarray(
                wq_c[:, D:].reshape(NK, P, D).transpose(1, 0, 2)
            ).astype(np.float16),
            "wkv": np.ascontiguousarray(
                wkv_c.reshape(NK, P, 2 * D).transpose(1, 0, 2)
            ).astype(np.float16),
            "wo": np.ascontiguousarray(
                wo_c.reshape(HQ, P, HID).transpose(1, 0, 2)
            ).astype(np.float16),
            "cosT": cosT.astype(np.float16),
            "sinT": sinT.astype(np.float16),
            "qw": qwv,
            "kw": kwv,
            "masks": masks,
        })
    return in_maps


def kernel(x, cos, sin, Wq, Wk, Wv, Wo, q_norm_w, k_norm_w):
    nc = get_nc()
    in_maps = make_in_maps(x, cos, sin, Wq, Wk, Wv, Wo, q_norm_w, k_norm_w)
    res = run_bass_kernel_spmd(nc, in_maps, core_ids=list(range(N_CORES)))
    acc = np.zeros((T, HID), dtype=np.float32)
    for c in range(N_CORES):
        acc += res.results[c]["out"]
    return acc.reshape(1, T, HID)


# revision 89
# speedup vs baseline: 1.0085x; 1.0085x over previous
"""GQA attention block (B=1, T=2048, HID=2048, NQ=16, NKV=8, D=128) on 8 TRN2
NeuronCores.

Sharding: tensor-parallel over heads. Core c owns q-heads {2c, 2c+1} and
kv-head c. Each core computes, from the full x:
  Q^T/K^T shards (d on partitions) and V in [token, d] layout (via a
  transposed projection, so no PE transposes)  ->  per-head RMSNorm + RoPE
  ->  causal softmax attention (no max-subtraction; scores are O(5) for
  RMS-normed q/k; diagonal blocks trimmed to the causal width)  ->  partial
  o_proj with Wo row-shard, written out fp16.
The 8 partial [T, HID] outputs are summed on the host (the row-parallel
"unshard" step).

All matmul operands are fp16 (full PE rate at any free-dim size; rel err
~5e-4, far under the 2e-2 gate -- fp8 would blow the budget); PSUM
accumulation stays fp32. Softmax denominators are accumulated on the vector
engine (fp16, 2x mode) and partition-all-reduced on the otherwise-idle Pool
engine, keeping the PE free of reduction matmuls; RMS statistics use the
same trick. Scheduling is a single tile scope, software-pipelined so no
phase barrier drains the machine:
  - iteration tr: q projections -> attention score phase for qr=tr-1 ->
    deferred RMS/RoPE chains (their Act Sqrt would otherwise head-of-line
    block the exps; a dummy exp then prefetches the Act table flip) -> k/v
    projections -> attention context phase,
  - o_proj is emitted as single-PSUM-tile units woven into the exp-starved
    score stretches (Act exp throughput is the binding resource there),
    with PSUM->SBUF fp16 copies split between DVE and Act,
  - the last query-range's attention weaves qr=2's context phase and v(3)
    in as extra PE filler, and normalizes per half so o_proj(3) starts
    progressively.
DMA: transfers are FIFO in desc-gen order, so startup-critical loads ride
the Act HWDGE queue in exact need-order while x0 drips on SP; out-writes
are batched per 128-row block (HWDGE desc-gen is a serial 625ns/op
resource).
"""

import sys
from collections import deque

sys.path.insert(0, "/opt/trn_rl_repo")

import numpy as np

import concourse.bass as bass  # noqa: F401  (bass must import before tile)
import concourse.bass_isa as bass_isa
import concourse.mybir as mybir
import concourse.tile as tile
from concourse import bacc
from concourse.bass_utils import run_bass_kernel_spmd

N_CORES = 8
T = 2048
HID = 2048
NQ, NKV, D = 16, 8, 128
HQ = NQ // N_CORES  # q heads per core = 2
EPS = 1e-6
SCALE = D**-0.5

P = 128
H = D // 2           # rope half
NK = HID // P        # 16 k-chunks for projections
NTR = T // 512       # 4 T-ranges of 512
NTT = T // P         # 16 T-tiles of 128

F32 = mybir.dt.float32
F16 = mybir.dt.float16
ACT_EXP = mybir.ActivationFunctionType.Exp
ACT_SQUARE = mybir.ActivationFunctionType.Square
ACT_SQRT = mybir.ActivationFunctionType.Sqrt
RED_ADD = bass_isa.ReduceOp.add


def build_nc():
    nc = bacc.Bacc("TRN2", target_bir_lowering=False, debug=False,
                   num_devices=N_CORES)

    # ---- DRAM tensors (names = in_map keys); all pre-arranged on host ----
    xt = nc.dram_tensor("xt", [P, NK, T], F16, kind="ExternalInput")
    wq0 = nc.dram_tensor("wq0", [P, NK, D], F16, kind="ExternalInput")
    wq1 = nc.dram_tensor("wq1", [P, NK, D], F16, kind="ExternalInput")
    wkv = nc.dram_tensor("wkv", [P, NK, 2 * D], F16, kind="ExternalInput")
    wo = nc.dram_tensor("wo", [P, HQ, HID], F16, kind="ExternalInput")
    cosT = nc.dram_tensor("cosT", [D, T], F16, kind="ExternalInput")
    sinT = nc.dram_tensor("sinT", [D, T], F16, kind="ExternalInput")
    qw = nc.dram_tensor("qw", [D, 1], F32, kind="ExternalInput")
    kw = nc.dram_tensor("kw", [D, 1], F32, kind="ExternalInput")
    masks = nc.dram_tensor("masks", [P, 4, 512], F16, kind="ExternalInput")
    out = nc.dram_tensor("out", [T, HID], F16, kind="ExternalOutput")

    with tile.TileContext(nc) as tc:
        with (
            tc.tile_pool(name="cst", bufs=1) as cst,
            tc.tile_pool(name="fin", bufs=1) as fin,
            tc.tile_pool(name="xtp", bufs=2) as xtp,
            tc.tile_pool(name="rawp", bufs=2) as rawp,
            tc.tile_pool(name="sqp", bufs=3) as sqp,
            tc.tile_pool(name="rmsp", bufs=2) as rmsp,
            tc.tile_pool(name="ropep", bufs=3) as ropep,
            tc.tile_pool(name="atp", bufs=36) as atp,
            tc.tile_pool(name="accp", bufs=3) as accp,
            tc.tile_pool(name="dnp", bufs=2) as dnp,
            tc.tile_pool(name="otp", bufs=3) as otp,
            tc.tile_pool(name="psA", bufs=2, space="PSUM") as psA,
            tc.tile_pool(name="psS", bufs=3, space="PSUM") as psS,
            tc.tile_pool(name="psO", bufs=3, space="PSUM") as psO,
        ):
            # ---------- constants / weights resident in SBUF ----------
            wq_sb = [cst.tile([P, NK, D], F16, name=f"wq{h}_sb")
                     for h in range(HQ)]
            wkv_sb = cst.tile([P, NK, 2 * D], F16)
            wo_sb = cst.tile([P, HQ, HID], F16)
            cos_sb = cst.tile([P, T], F16)
            sin_sb = cst.tile([P, T], F16)   # pre-rolled, first half negated
            masks_sb = cst.tile([P, 4, 512], F16)
            qw_sb = cst.tile([P, 1], F32)
            kw_sb = cst.tile([P, 1], F32)
            eps_sb = cst.tile([P, 1], F32)
            nc.gpsimd.memset(eps_sb[:], EPS)
            dmy = cst.tile([1, 1], F32)
            nc.gpsimd.memset(dmy[:], 0.0)
            dmy_o = cst.tile([1, 1], F16)

            def preload_exp():
                # A do-nothing exp: forces the Act function-table switch
                # (1.3us) to happen NOW, while Act is idle, instead of right
                # in front of the first real exp of the attention phase.
                nc.scalar.activation(dmy_o[:], dmy[:], ACT_EXP)

            # DMA transfers run FIFO in desc-gen order, and desc-gen follows
            # per-queue program order -- so everything startup-critical rides
            # the Act HWDGE queue in exact need-order (weights, then x1 and
            # trig tables interleaved), x0/x2/x3/out ride SP, and wo (needed
            # only by o_proj at ~40us) is emitted inside iteration 1 on the
            # Pool SWDGE queue so it cannot jump the early queue.
            # x0's first drip goes before the weights so its desc-gen (and
            # transfer) is first in the FIFO; the rest interleave by need.
            xch01 = []
            for tr01 in range(2):
                xch_t = xtp.tile([P, NK, 512], F16, name="xch")
                t0 = slice(tr01 * 512, (tr01 + 1) * 512)
                if tr01 == 0:
                    # drip x0 in 4 chunks so the first projection group can
                    # chew k-chunks while the rest stream in
                    for ks in (slice(0, 2), slice(2, 5), slice(5, 10),
                               slice(10, 16)):
                        nc.sync.dma_start(xch_t[:, ks, :], xt[:, ks, t0])
                    nc.scalar.dma_start(wq_sb[0][:], wq0[:])
                    nc.scalar.dma_start(qw_sb[:], qw[:])
                    nc.scalar.dma_start(kw_sb[:], kw[:])
                    nc.scalar.dma_start(wq_sb[1][:], wq1[:])
                    nc.scalar.dma_start(wkv_sb[:], wkv[:])
                else:
                    nc.scalar.dma_start(xch_t[:, 0:6, :], xt[:, 0:6, t0])
                    nc.scalar.dma_start(xch_t[:, 6:11, :],
                                        xt[:, 6:11, t0])
                    nc.scalar.dma_start(xch_t[:, 11:, :], xt[:, 11:, t0])
                    nc.scalar.dma_start(cos_sb[:], cosT[:])
                    nc.scalar.dma_start(sin_sb[:], sinT[:])
                    nc.scalar.dma_start(masks_sb[:], masks[:])
                    nc.scalar.dma_start(wo_sb[:], wo[:])
                xch01.append(xch_t)

            # final (post RMS+RoPE) activations, fp16
            qT = [fin.tile([P, T], F16, name=f"qT{h}") for h in range(HQ)]
            kT = fin.tile([P, T], F16)
            vnat = fin.tile([P, NTT, D], F16)
            ctxT = [
                [fin.tile([P, 512], F16, name=f"ctxT{h}_{qr}")
                 for qr in range(NTR)]
                for h in range(HQ)
            ]

            COLS = {
                "q0": (qT[0], (lambda: wq_sb[0]), 0, qw_sb),
                "q1": (qT[1], (lambda: wq_sb[1]), 0, qw_sb),
                "k": (kT, (lambda: wkv_sb), 0, kw_sb),
            }

            def emit_proj_mm(tr, xch, name):
                """Projection matmul group + psum evacuation (Act Copy +
                Square only -- both table-neutral, so they never delay the
                attention exps that follow on the Act queue)."""
                _, w_fn, off, _ = COLS[name]
                w_sb = w_fn()
                ps = psA.tile([P, 512], F32, name="psA_t")
                for k in range(NK):
                    nc.tensor.matmul(
                        ps[:], w_sb[:, k, off:off + D], xch[:, k, :],
                        start=(k == 0), stop=(k == NK - 1),
                    )
                sq = sqp.tile([P, 512], F16, name="sq")
                nc.scalar.activation(sq[:], ps[:], ACT_SQUARE)
                raw = rawp.tile([P, 512], F32, name=f"raw_{name}")
                nc.scalar.copy(raw[:], ps[:])
                return raw, sq

            def emit_bchain(tr, name, raw, sq):
                """Deferred RMS+RoPE chain (contains the Act Sqrt, so it is
                emitted AFTER the attention score phase: its table flip then
                lands in Act slack, not in front of the exps)."""
                ts = slice(tr * 512, (tr + 1) * 512)
                dst, _, _, nrm_w = COLS[name]
                ssum_b = rmsp.tile([P, 512], F32, name="ssum_b")
                nc.gpsimd.partition_all_reduce(ssum_b[:], sq[:],
                                               channels=P,
                                               reduce_op=RED_ADD)
                rstd_b = rmsp.tile([P, 512], F32, name="rstd_b")
                nc.scalar.activation(rstd_b[:], ssum_b[:], ACT_SQRT,
                                     scale=1.0 / D, bias=eps_sb[:])
                rinv_b = rmsp.tile([P, 512], F32, name="rinv_b")
                nc.vector.reciprocal_approx_fast(rinv_b[:], rstd_b[:])
                nq = ropep.tile([P, 512], F16, name="nq")
                nc.vector.scalar_tensor_tensor(
                    nq[:], raw[:], nrm_w[:], rinv_b[:],
                    mybir.AluOpType.mult, mybir.AluOpType.mult,
                )
                pc = ropep.tile([P, 512], F16, name="pc")
                nc.vector.tensor_mul(pc[:], nq[:], cos_sb[:, ts])
                psn = ropep.tile([P, 512], F16, name="psn")
                nc.vector.tensor_mul(psn[0:H, :], nq[H:D, :],
                                     sin_sb[H:D, ts])
                nc.vector.tensor_mul(psn[H:D, :], nq[0:H, :],
                                     sin_sb[0:H, ts])
                nc.vector.tensor_add(dst[:, ts], pc[:], psn[:])

            def emit_proj_v(tr, xch):
                # --- v : out [128 t, 128 d] per t-tile, no transpose
                vps = psA.tile([P, 512], F32, name="psA_t")
                for j in range(4):
                    for k in range(NK):
                        nc.tensor.matmul(
                            vps[:, j * P:(j + 1) * P],
                            xch[:, k, j * P:(j + 1) * P],
                            wkv_sb[:, k, D:2 * D],
                            start=(k == 0), stop=(k == NK - 1),
                        )
                nc.vector.tensor_copy(vnat[:, 4 * tr:4 * tr + 4, :],
                                      vps[:])

            def emit_scores(h, qr, mids=(), filler=None, filler_from=0):
                """Score matmuls + exp + causal mask + fp16 denominator
                accumulation for one (head, query-range). Returns (ats,
                qoffs, acc). mids: (st, callback) pairs weaving in big PE
                blocks; filler: a deque of small PE units (single o_proj
                tiles) popped every other st to soak up exp latency."""
                n_st = 4 * (qr + 1)
                ats, qoffs = [], []
                acc = accp.tile([P, 512], F16, name="acc")
                for st in range(n_st):
                    for st_m, cb in mids:
                        if st == st_m:
                            cb()
                    if filler and st % 2 == 1 and st >= filler_from:
                        filler.popleft()()
                    j = st - 4 * qr
                    qoff = max(0, 128 * j) if j >= 0 else 0
                    s_ps = psS.tile([P, 512], F32, name="s_ps")
                    nc.tensor.matmul(
                        s_ps[:, qoff:], kT[:, st * P:(st + 1) * P],
                        qT[h][:, qr * 512 + qoff:(qr + 1) * 512],
                        start=True, stop=True,
                    )
                    at = atp.tile([P, 512], F16, name="at")
                    nc.scalar.activation(at[:, qoff:], s_ps[:, qoff:],
                                         ACT_EXP, scale=SCALE)
                    if j >= 0:
                        nc.vector.tensor_mul(at[:, qoff:], at[:, qoff:],
                                             masks_sb[:, j, qoff:])
                    # accumulate into acc on DVE (fp16 2x mode)
                    if st == 1:
                        if qoffs[0] == 0 and qoff == 0:
                            nc.vector.tensor_add(acc[:], ats[0][:], at[:])
                        else:  # qr == 0: at0 full, at1 starts at 128
                            nc.vector.tensor_copy(acc[:, :qoff],
                                                  ats[0][:, :qoff])
                            nc.vector.tensor_add(acc[:, qoff:],
                                                 ats[0][:, qoff:],
                                                 at[:, qoff:])
                    elif st >= 2:
                        nc.vector.tensor_add(acc[:, qoff:], acc[:, qoff:],
                                             at[:, qoff:])
                    ats.append(at)
                    qoffs.append(qoff)
                return ats, qoffs, acc

            def emit_ctx(h, qr, ats, qoffs, acc, filler=None):
                """Context matmuls + denominator all-reduce + normalize.
                For the final query-range the normalize runs per 128-query
                quarter so o_proj(3) t-tiles can start progressively."""
                n_st = 4 * (qr + 1)
                ctx_ps = psO.tile([P, 512], F32, name="psO_t")
                for st in range(n_st):
                    if filler and st % 2 == 1:
                        filler.popleft()()
                    qoff = qoffs[st]
                    nc.tensor.matmul(
                        ctx_ps[:, qoff:], vnat[:, st, :], ats[st][:, qoff:],
                        start=(st == 0), stop=(st == n_st - 1),
                    )
                denom_b = dnp.tile([P, 512], F32, name="denom_b")
                rb = dnp.tile([P, 512], F32, name="rb")
                nc.gpsimd.partition_all_reduce(denom_b[:], acc[:],
                                               channels=P,
                                               reduce_op=RED_ADD)
                halves = (
                    [slice(0, 256), slice(256, 512)]
                    if qr == NTR - 1 else [slice(0, 512)]
                )
                for qs4 in halves:
                    nc.vector.reciprocal_approx_fast(rb[:, qs4],
                                                     denom_b[:, qs4])
                    nc.vector.tensor_mul(ctxT[h][qr][:, qs4],
                                         ctx_ps[:, qs4], rb[:, qs4])

            oproj_n = [0]

            def emit_oproj(qr, tts, act_every=4, split_dma=False):
                """o_proj for t-tiles tts of query-range qr. PSUM -> fp16
                SBUF copies (GPSIMD can't read PSUM on hw) split between DVE
                and Act (every act_every-th on Act); the 4 n-range tiles of
                one t-tile stage into one [P, HID] buffer so a single
                batched DMA covers the whole row block (HWDGE desc-gen is a
                serial 625ns/op resource -- 16 DMAs, not 64). split_dma
                switches the last tile back to per-n-range DMAs so the
                drain tail is fine-grained."""
                for tt in tts:
                    off = (tt % 4) * P
                    ot = otp.tile([P, HID], F16, name="ot")
                    last = split_dma and tt == tts[-1]
                    for nr in range(NTR):
                        ns = slice(nr * 512, (nr + 1) * 512)
                        ps = psO.tile([P, 512], F32, name="psO_t")
                        for h in range(HQ):
                            nc.tensor.matmul(
                                ps[:], ctxT[h][qr][:, off:off + P],
                                wo_sb[:, h, ns],
                                start=(h == 0), stop=(h == HQ - 1),
                            )
                        use_act = oproj_n[0] % act_every == act_every - 1
                        if use_act:
                            nc.scalar.copy(ot[:, ns], ps[:])
                        else:
                            nc.vector.tensor_copy(ot[:, ns], ps[:])
                        oproj_n[0] += 1
                        if last:
                            nc.sync.dma_start(
                                out[tt * P:(tt + 1) * P, ns], ot[:, ns])
                    if not last:
                        nc.sync.dma_start(out[tt * P:(tt + 1) * P, :], ot[:])

            def oproj_units(qr, tts, act_every=4):
                """Per-(t-tile, n-range) o_proj emission units for fine
                weaving into exp-starved stretches."""
                units, ots = [], {}
                for tt in tts:
                    for nr in range(NTR):
                        def u(tt=tt, nr=nr):
                            off = (tt % 4) * P
                            if nr == 0:
                                ots[tt] = otp.tile([P, HID], F16, name="ot")
                            ot = ots[tt]
                            ns = slice(nr * 512, (nr + 1) * 512)
                            ps = psO.tile([P, 512], F32, name="psO_t")
                            for h in range(HQ):
                                nc.tensor.matmul(
                                    ps[:], ctxT[h][qr][:, off:off + P],
                                    wo_sb[:, h, ns],
                                    start=(h == 0), stop=(h == HQ - 1),
                                )
                            use_act = (oproj_n[0] % act_every
                                       == act_every - 1)
                            if use_act:
                                nc.scalar.copy(ot[:, ns], ps[:])
                            else:
                                nc.vector.tensor_copy(ot[:, ns], ps[:])
                            oproj_n[0] += 1
                            if nr == NTR - 1:
                                nc.sync.dma_start(
                                    out[tt * P:(tt + 1) * P, :], ot[:])
                        units.append(u)
                return units

            def emit_attn_a(qr, filler, extra=()):
                """Score phase for both heads of query-range qr; filler
                units (o_proj tiles of qr-1) pop every other st."""
                ex = list(extra)
                if ex:  # last-qr: qr-2's ctx blocks lead (they are the
                    # early filler and the units depend on their output)
                    m0 = [(1, ex[0]), (4, ex[1])]
                    m1 = [(2, ex[2])]
                    a0 = emit_scores(0, qr, mids=m0, filler=filler,
                                     filler_from=6)
                    a1 = emit_scores(1, qr, mids=m1, filler=filler)
                else:
                    a0 = emit_scores(0, qr, filler=filler)
                    a1 = emit_scores(1, qr, filler=filler)
                return a0, a1

            def emit_attn_b(qr, a0, a1, filler=None):
                """Context phase for both heads of query-range qr; drains
                any remaining filler units at the end."""
                emit_ctx(0, qr, *a0, filler=filler)
                emit_ctx(1, qr, *a1, filler=filler)
                while filler:
                    filler.popleft()()

            # ==================== main pipelined loop ====================
            for tr in range(NTR):
                ts = slice(tr * 512, (tr + 1) * 512)
                if tr < 2:
                    xch = xch01[tr]
                else:
                    xch = xtp.tile([P, NK, 512], F16, name="xch")
                    for kg in range(2):
                        ks = slice(kg * 8, (kg + 1) * 8)
                        nc.sync.dma_start(xch[:, ks, :], xt[:, ks, ts])
                filler = None
                if tr >= 2:
                    filler = deque(oproj_units(
                        tr - 2, list(range(4 * (tr - 2), 4 * (tr - 1)))))
                aa = emit_attn_a(tr - 1, filler) if tr >= 1 else None
                rq0 = emit_proj_mm(tr, xch, "q0")
                rq1 = emit_proj_mm(tr, xch, "q1")
                emit_bchain(tr, "q0", *rq0)
                emit_bchain(tr, "q1", *rq1)
                rk = emit_proj_mm(tr, xch, "k")
                emit_bchain(tr, "k", *rk)
                preload_exp()
                if tr < NTR - 1:
                    emit_proj_v(tr, xch)
                else:
                    xch_last = xch  # v(3) woven into attn(3)'s score phase
                if aa is not None:
                    if tr < NTR - 1:
                        emit_attn_b(tr - 1, *aa, filler=filler)
                    else:
                        # last iteration: qr=2's ctx phase is woven into
                        # attn(3)'s exp-starved score phase instead
                        aa_prev = aa
                        filler_prev = filler
            # drain any oproj(1) units not yet emitted, then the final
            # region: attn(3) with ctx(2)/v(3) as leading filler and
            # oproj(2) units woven through, then oproj(3).
            while filler_prev:
                filler_prev.popleft()()
            filler = deque(oproj_units(
                NTR - 2, list(range(4 * (NTR - 2), 4 * (NTR - 1)))))
            a2_0, a2_1 = aa_prev
            aa = emit_attn_a(
                NTR - 1, filler,
                extra=(lambda: emit_ctx(0, NTR - 2, *a2_0),
                       lambda: emit_ctx(1, NTR - 2, *a2_1),
                       lambda: emit_proj_v(NTR - 1, xch_last)),
            )
            emit_attn_b(NTR - 1, *aa, filler=filler)
            emit_oproj(NTR - 1, list(range(4 * (NTR - 1), 4 * NTR)),
                       act_every=2, split_dma=True)

    nc.compile()
    return nc


_NC_CACHE = None


def get_nc():
    global _NC_CACHE
    if _NC_CACHE is None:
        _NC_CACHE = build_nc()
    return _NC_CACHE


def make_in_maps(x, cos, sin, Wq, Wk, Wv, Wo, q_norm_w, k_norm_w):
    x = np.asarray(x, dtype=np.float32).reshape(T, HID)
    # xt: [P, NK, T] fp16, HID index = k*P + p
    xt = np.ascontiguousarray(
        x.T.reshape(NK, P, T).transpose(1, 0, 2).astype(np.float16)
    )
    cosT = np.ascontiguousarray(np.asarray(cos, np.float32).T)
    # sin, transposed, first half negated, then rolled by 64 partitions:
    # psn = rot_half-mul uses sin_sb[H:] for dst[:H] (needs -sin[:H]) and
    # sin_sb[:H] for dst[H:] (needs +sin[H:]); single add then applies RoPE.
    sin_t = np.asarray(sin, np.float32).T.copy()
    sin_t[:H] *= -1.0
    sinT = np.ascontiguousarray(np.roll(sin_t, H, axis=0))
    qwv = np.ascontiguousarray(np.asarray(q_norm_w, np.float32).reshape(D, 1))
    kwv = np.ascontiguousarray(np.asarray(k_norm_w, np.float32).reshape(D, 1))
    si = np.arange(P)[:, None, None]
    jj = np.arange(4)[None, :, None]
    qi = np.arange(512)[None, None, :]
    masks = (si + P * jj <= qi).astype(np.float16)
    Wq = np.asarray(Wq, np.float32)
    Wk = np.asarray(Wk, np.float32)
    Wv = np.asarray(Wv, np.float32)
    Wo = np.asarray(Wo, np.float32)
    in_maps = []
    for c in range(N_CORES):
        wq_c = Wq[:, c * HQ * D:(c + 1) * HQ * D]      # [HID, 256]
        wk_c = Wk[:, c * D:(c + 1) * D]                # [HID, 128]
        wv_c = Wv[:, c * D:(c + 1) * D]                # [HID, 128]
        wkv_c = np.concatenate([wk_c, wv_c], axis=1)   # [HID, 256]
        wo_c = Wo[c * HQ * D:(c + 1) * HQ * D, :]      # [256, HID]
        in_maps.append({
            "xt": xt,
            "wq0": np.ascontiguousarray(
                wq_c[:, 0:D].reshape(NK, P, D).transpose(1, 0, 2)
            ).astype(np.float16),
            "wq1": np.ascontiguousarray(
                wq_c[:, D:].reshape(NK, P, D).transpose(1, 0, 2)
            ).astype(np.float16),
            "wkv": np.ascontiguousarray(
                wkv_c.reshape(NK, P, 2 * D).transpose(1, 0, 2)
            ).astype(np.float16),
            "wo": np.ascontiguousarray(
                wo_c.reshape(HQ, P, HID).transpose(1, 0, 2)
            ).astype(np.float16),
            "cosT": cosT.astype(np.float16),
            "sinT": sinT.astype(np.float16),
            "qw": qwv,
            "kw": kwv,
            "masks": masks,
        })
    return in_maps


def kernel(x, cos, sin, Wq, Wk, Wv, Wo, q_norm_w, k_norm_w):
    nc = get_nc()
    in_maps = make_in_maps(x, cos, sin, Wq, Wk, Wv, Wo, q_norm_w, k_norm_w)
    res = run_bass_kernel_spmd(nc, in_maps, core_ids=list(range(N_CORES)))
    acc = np.zeros((T, HID), dtype=np.float32)
    for c in range(N_CORES):
        acc += res.results[c]["out"]
    return acc.reshape(1, T, HID)


# revision 100
# speedup vs baseline: 1.0134x; 1.0049x over previous
"""GQA attention block (B=1, T=2048, HID=2048, NQ=16, NKV=8, D=128) on 8 TRN2
NeuronCores.

Sharding: tensor-parallel over heads. Core c owns q-heads {2c, 2c+1} and
kv-head c. Each core computes, from the full x:
  Q^T/K^T shards (d on partitions) and V in [token, d] layout (via a
  transposed projection, so no PE transposes)  ->  per-head RMSNorm + RoPE
  ->  causal softmax attention (no max-subtraction; scores are O(5) for
  RMS-normed q/k; diagonal blocks trimmed to the causal width)  ->  partial
  o_proj with Wo row-shard, written out fp16.
The 8 partial [T, HID] outputs are summed on the host (the row-parallel
"unshard" step).

All matmul operands are fp16 (full PE rate at any free-dim size; rel err
~5e-4, far under the 2e-2 gate -- fp8 would blow the budget); PSUM
accumulation stays fp32. Softmax denominators are accumulated on the vector
engine (fp16, 2x mode) and partition-all-reduced on the otherwise-idle Pool
engine, keeping the PE free of reduction matmuls; RMS statistics use the
same trick. Scheduling is a single tile scope, software-pipelined so no
phase barrier drains the machine:
  - iteration tr: attention score phase for qr=tr-1 (its kT/qT completed
    mid-previous-iteration; the q projections behind it absorb the exp
    drain) -> q projections -> deferred RMS/RoPE chains (their Act Sqrt
    would otherwise head-of-line block the exps; a dummy exp then
    prefetches the Act table flip) -> k/v projections -> attention context
    phase,
  - o_proj is emitted as single-PSUM-tile units woven into the exp-starved
    score stretches (Act exp throughput is the binding resource there),
    with PSUM->SBUF fp16 copies split between DVE and Act,
  - softmax normalization runs per 256-query half so o_proj units' ctxT
    dependencies resolve early; the last query-range's attention weaves
    qr=2's context phase and v(3) in as extra PE filler.
DMA: transfers are FIFO in desc-gen order, so startup-critical loads ride
the Act HWDGE queue in exact need-order while x0 drips on SP; out-writes
are batched per 128-row block (HWDGE desc-gen is a serial 625ns/op
resource).
"""

import sys
from collections import deque

sys.path.insert(0, "/opt/trn_rl_repo")

import numpy as np

import concourse.bass as bass  # noqa: F401  (bass must import before tile)
import concourse.bass_isa as bass_isa
import concourse.mybir as mybir
import concourse.tile as tile
from concourse import bacc
from concourse.bass_utils import run_bass_kernel_spmd

N_CORES = 8
T = 2048
HID = 2048
NQ, NKV, D = 16, 8, 128
HQ = NQ // N_CORES  # q heads per core = 2
EPS = 1e-6
SCALE = D**-0.5

P = 128
H = D // 2           # rope half
NK = HID // P        # 16 k-chunks for projections
NTR = T // 512       # 4 T-ranges of 512
NTT = T // P         # 16 T-tiles of 128

F32 = mybir.dt.float32
F16 = mybir.dt.float16
ACT_EXP = mybir.ActivationFunctionType.Exp
ACT_SQUARE = mybir.ActivationFunctionType.Square
ACT_SQRT = mybir.ActivationFunctionType.Sqrt
RED_ADD = bass_isa.ReduceOp.add


def build_nc():
    nc = bacc.Bacc("TRN2", target_bir_lowering=False, debug=False,
                   num_devices=N_CORES)

    # ---- DRAM tensors (names = in_map keys); all pre-arranged on host ----
    xt = nc.dram_tensor("xt", [P, NK, T], F16, kind="ExternalInput")
    wq0 = nc.dram_tensor("wq0", [P, NK, D], F16, kind="ExternalInput")
    wq1 = nc.dram_tensor("wq1", [P, NK, D], F16, kind="ExternalInput")
    wkv = nc.dram_tensor("wkv", [P, NK, 2 * D], F16, kind="ExternalInput")
    wo = nc.dram_tensor("wo", [P, HQ, HID], F16, kind="ExternalInput")
    cosT = nc.dram_tensor("cosT", [D, T], F16, kind="ExternalInput")
    sinT = nc.dram_tensor("sinT", [D, T], F16, kind="ExternalInput")
    qw = nc.dram_tensor("qw", [D, 1], F32, kind="ExternalInput")
    kw = nc.dram_tensor("kw", [D, 1], F32, kind="ExternalInput")
    masks = nc.dram_tensor("masks", [P, 4, 512], F16, kind="ExternalInput")
    out = nc.dram_tensor("out", [T, HID], F16, kind="ExternalOutput")

    with tile.TileContext(nc) as tc:
        with (
            tc.tile_pool(name="cst", bufs=1) as cst,
            tc.tile_pool(name="fin", bufs=1) as fin,
            tc.tile_pool(name="xtp", bufs=2) as xtp,
            tc.tile_pool(name="rawp", bufs=2) as rawp,
            tc.tile_pool(name="sqp", bufs=3) as sqp,
            tc.tile_pool(name="rmsp", bufs=2) as rmsp,
            tc.tile_pool(name="ropep", bufs=3) as ropep,
            tc.tile_pool(name="atp", bufs=36) as atp,
            tc.tile_pool(name="accp", bufs=3) as accp,
            tc.tile_pool(name="dnp", bufs=2) as dnp,
            tc.tile_pool(name="otp", bufs=3) as otp,
            tc.tile_pool(name="psA", bufs=2, space="PSUM") as psA,
            tc.tile_pool(name="psS", bufs=3, space="PSUM") as psS,
            tc.tile_pool(name="psO", bufs=3, space="PSUM") as psO,
        ):
            # ---------- constants / weights resident in SBUF ----------
            wq_sb = [cst.tile([P, NK, D], F16, name=f"wq{h}_sb")
                     for h in range(HQ)]
            wkv_sb = cst.tile([P, NK, 2 * D], F16)
            wo_sb = cst.tile([P, HQ, HID], F16)
            cos_sb = cst.tile([P, T], F16)
            sin_sb = cst.tile([P, T], F16)   # pre-rolled, first half negated
            masks_sb = cst.tile([P, 4, 512], F16)
            qw_sb = cst.tile([P, 1], F32)
            kw_sb = cst.tile([P, 1], F32)
            eps_sb = cst.tile([P, 1], F32)
            nc.gpsimd.memset(eps_sb[:], EPS)
            dmy = cst.tile([1, 1], F32)
            nc.gpsimd.memset(dmy[:], 0.0)
            dmy_o = cst.tile([1, 1], F16)

            def preload_exp():
                # A do-nothing exp: forces the Act function-table switch
                # (1.3us) to happen NOW, while Act is idle, instead of right
                # in front of the first real exp of the attention phase.
                nc.scalar.activation(dmy_o[:], dmy[:], ACT_EXP)

            # DMA transfers run FIFO in desc-gen order, and desc-gen follows
            # per-queue program order -- so everything startup-critical rides
            # the Act HWDGE queue in exact need-order (weights, then x1 and
            # trig tables interleaved), x0/x2/x3/out ride SP, and wo (needed
            # only by o_proj at ~40us) is emitted inside iteration 1 on the
            # Pool SWDGE queue so it cannot jump the early queue.
            # x0's first drip goes before the weights so its desc-gen (and
            # transfer) is first in the FIFO; the rest interleave by need.
            xch01 = []
            for tr01 in range(2):
                xch_t = xtp.tile([P, NK, 512], F16, name="xch")
                t0 = slice(tr01 * 512, (tr01 + 1) * 512)
                if tr01 == 0:
                    # drip x0 in 4 chunks so the first projection group can
                    # chew k-chunks while the rest stream in
                    for ks in (slice(0, 2), slice(2, 5), slice(5, 10),
                               slice(10, 16)):
                        nc.sync.dma_start(xch_t[:, ks, :], xt[:, ks, t0])
                    nc.scalar.dma_start(wq_sb[0][:], wq0[:])
                    nc.scalar.dma_start(qw_sb[:], qw[:])
                    nc.scalar.dma_start(kw_sb[:], kw[:])
                    nc.scalar.dma_start(wq_sb[1][:], wq1[:])
                    nc.scalar.dma_start(wkv_sb[:], wkv[:])
                else:
                    nc.scalar.dma_start(xch_t[:, 0:6, :], xt[:, 0:6, t0])
                    nc.scalar.dma_start(xch_t[:, 6:11, :],
                                        xt[:, 6:11, t0])
                    nc.scalar.dma_start(xch_t[:, 11:, :], xt[:, 11:, t0])
                    nc.scalar.dma_start(cos_sb[:], cosT[:])
                    nc.scalar.dma_start(sin_sb[:], sinT[:])
                    nc.scalar.dma_start(masks_sb[:], masks[:])
                    nc.scalar.dma_start(wo_sb[:], wo[:])
                xch01.append(xch_t)

            # final (post RMS+RoPE) activations, fp16
            qT = [fin.tile([P, T], F16, name=f"qT{h}") for h in range(HQ)]
            kT = fin.tile([P, T], F16)
            vnat = fin.tile([P, NTT, D], F16)
            ctxT = [
                [fin.tile([P, 512], F16, name=f"ctxT{h}_{qr}")
                 for qr in range(NTR)]
                for h in range(HQ)
            ]

            COLS = {
                "q0": (qT[0], (lambda: wq_sb[0]), 0, qw_sb),
                "q1": (qT[1], (lambda: wq_sb[1]), 0, qw_sb),
                "k": (kT, (lambda: wkv_sb), 0, kw_sb),
            }

            def emit_proj_mm(tr, xch, name):
                """Projection matmul group + psum evacuation (Act Copy +
                Square only -- both table-neutral, so they never delay the
                attention exps that follow on the Act queue)."""
                _, w_fn, off, _ = COLS[name]
                w_sb = w_fn()
                ps = psA.tile([P, 512], F32, name="psA_t")
                for k in range(NK):
                    nc.tensor.matmul(
                        ps[:], w_sb[:, k, off:off + D], xch[:, k, :],
                        start=(k == 0), stop=(k == NK - 1),
                    )
                sq = sqp.tile([P, 512], F16, name="sq")
                nc.scalar.activation(sq[:], ps[:], ACT_SQUARE)
                raw = rawp.tile([P, 512], F32, name=f"raw_{name}")
                nc.scalar.copy(raw[:], ps[:])
                return raw, sq

            def emit_bchain(tr, name, raw, sq):
                """Deferred RMS+RoPE chain (contains the Act Sqrt, so it is
                emitted AFTER the attention score phase: its table flip then
                lands in Act slack, not in front of the exps)."""
                ts = slice(tr * 512, (tr + 1) * 512)
                dst, _, _, nrm_w = COLS[name]
                ssum_b = rmsp.tile([P, 512], F32, name="ssum_b")
                nc.gpsimd.partition_all_reduce(ssum_b[:], sq[:],
                                               channels=P,
                                               reduce_op=RED_ADD)
                rstd_b = rmsp.tile([P, 512], F32, name="rstd_b")
                nc.scalar.activation(rstd_b[:], ssum_b[:], ACT_SQRT,
                                     scale=1.0 / D, bias=eps_sb[:])
                rinv_b = rmsp.tile([P, 512], F32, name="rinv_b")
                nc.vector.reciprocal_approx_fast(rinv_b[:], rstd_b[:])
                nq = ropep.tile([P, 512], F16, name="nq")
                nc.vector.scalar_tensor_tensor(
                    nq[:], raw[:], nrm_w[:], rinv_b[:],
                    mybir.AluOpType.mult, mybir.AluOpType.mult,
                )
                pc = ropep.tile([P, 512], F16, name="pc")
                nc.vector.tensor_mul(pc[:], nq[:], cos_sb[:, ts])
                psn = ropep.tile([P, 512], F16, name="psn")
                nc.vector.tensor_mul(psn[0:H, :], nq[H:D, :],
                                     sin_sb[H:D, ts])
                nc.vector.tensor_mul(psn[H:D, :], nq[0:H, :],
                                     sin_sb[0:H, ts])
                nc.vector.tensor_add(dst[:, ts], pc[:], psn[:])

            def emit_proj_v(tr, xch):
                # --- v : out [128 t, 128 d] per t-tile, no transpose
                vps = psA.tile([P, 512], F32, name="psA_t")
                for j in range(4):
                    for k in range(NK):
                        nc.tensor.matmul(
                            vps[:, j * P:(j + 1) * P],
                            xch[:, k, j * P:(j + 1) * P],
                            wkv_sb[:, k, D:2 * D],
                            start=(k == 0), stop=(k == NK - 1),
                        )
                nc.vector.tensor_copy(vnat[:, 4 * tr:4 * tr + 4, :],
                                      vps[:])

            def emit_scores(h, qr, mids=(), filler=None, filler_from=0):
                """Score matmuls + exp + causal mask + fp16 denominator
                accumulation for one (head, query-range). Returns (ats,
                qoffs, acc). mids: (st, callback) pairs weaving in big PE
                blocks; filler: a deque of small PE units (single o_proj
                tiles) popped every other st to soak up exp latency."""
                n_st = 4 * (qr + 1)
                ats, qoffs = [], []
                acc = accp.tile([P, 512], F16, name="acc")
                for st in range(n_st):
                    for st_m, cb in mids:
                        if st == st_m:
                            cb()
                    if filler and st % 2 == 1 and st >= filler_from:
                        filler.popleft()()
                    j = st - 4 * qr
                    qoff = max(0, 128 * j) if j >= 0 else 0
                    s_ps = psS.tile([P, 512], F32, name="s_ps")
                    nc.tensor.matmul(
                        s_ps[:, qoff:], kT[:, st * P:(st + 1) * P],
                        qT[h][:, qr * 512 + qoff:(qr + 1) * 512],
                        start=True, stop=True,
                    )
                    at = atp.tile([P, 512], F16, name="at")
                    nc.scalar.activation(at[:, qoff:], s_ps[:, qoff:],
                                         ACT_EXP, scale=SCALE)
                    if j >= 0:
                        nc.vector.tensor_mul(at[:, qoff:], at[:, qoff:],
                                             masks_sb[:, j, qoff:])
                    # accumulate into acc on DVE (fp16 2x mode)
                    if st == 1:
                        if qoffs[0] == 0 and qoff == 0:
                            nc.vector.tensor_add(acc[:], ats[0][:], at[:])
                        else:  # qr == 0: at0 full, at1 starts at 128
                            nc.vector.tensor_copy(acc[:, :qoff],
                                                  ats[0][:, :qoff])
                            nc.vector.tensor_add(acc[:, qoff:],
                                                 ats[0][:, qoff:],
                                                 at[:, qoff:])
                    elif st >= 2:
                        nc.vector.tensor_add(acc[:, qoff:], acc[:, qoff:],
                                             at[:, qoff:])
                    ats.append(at)
                    qoffs.append(qoff)
                return ats, qoffs, acc

            def emit_ctx(h, qr, ats, qoffs, acc, filler=None):
                """Context matmuls + denominator all-reduce + normalize.
                For the final query-range the normalize runs per 128-query
                quarter so o_proj(3) t-tiles can start progressively."""
                n_st = 4 * (qr + 1)
                ctx_ps = psO.tile([P, 512], F32, name="psO_t")
                for st in range(n_st):
                    if filler and st % 2 == 1:
                        filler.popleft()()
                    qoff = qoffs[st]
                    nc.tensor.matmul(
                        ctx_ps[:, qoff:], vnat[:, st, :], ats[st][:, qoff:],
                        start=(st == 0), stop=(st == n_st - 1),
                    )
                denom_b = dnp.tile([P, 512], F32, name="denom_b")
                rb = dnp.tile([P, 512], F32, name="rb")
                nc.gpsimd.partition_all_reduce(denom_b[:], acc[:],
                                               channels=P,
                                               reduce_op=RED_ADD)
                for qs4 in (slice(0, 256), slice(256, 512)):
                    nc.vector.reciprocal_approx_fast(rb[:, qs4],
                                                     denom_b[:, qs4])
                    nc.vector.tensor_mul(ctxT[h][qr][:, qs4],
                                         ctx_ps[:, qs4], rb[:, qs4])

            oproj_n = [0]

            def emit_oproj(qr, tts, act_every=4, split_dma=False):
                """o_proj for t-tiles tts of query-range qr. PSUM -> fp16
                SBUF copies (GPSIMD can't read PSUM on hw) split between DVE
                and Act (every act_every-th on Act); the 4 n-range tiles of
                one t-tile stage into one [P, HID] buffer so a single
                batched DMA covers the whole row block (HWDGE desc-gen is a
                serial 625ns/op resource -- 16 DMAs, not 64). split_dma
                switches the last tile back to per-n-range DMAs so the
                drain tail is fine-grained."""
                for tt in tts:
                    off = (tt % 4) * P
                    ot = otp.tile([P, HID], F16, name="ot")
                    last = split_dma and tt == tts[-1]
                    for nr in range(NTR):
                        ns = slice(nr * 512, (nr + 1) * 512)
                        ps = psO.tile([P, 512], F32, name="psO_t")
                        for h in range(HQ):
                            nc.tensor.matmul(
                                ps[:], ctxT[h][qr][:, off:off + P],
                                wo_sb[:, h, ns],
                                start=(h == 0), stop=(h == HQ - 1),
                            )
                        use_act = oproj_n[0] % act_every == act_every - 1
                        if use_act:
                            nc.scalar.copy(ot[:, ns], ps[:])
                        else:
                            nc.vector.tensor_copy(ot[:, ns], ps[:])
                        oproj_n[0] += 1
                        if last:
                            nc.sync.dma_start(
                                out[tt * P:(tt + 1) * P, ns], ot[:, ns])
                    if not last:
                        nc.sync.dma_start(out[tt * P:(tt + 1) * P, :], ot[:])

            def oproj_units(qr, tts, act_every=4):
                """Per-(t-tile, n-range) o_proj emission units for fine
                weaving into exp-starved stretches."""
                units, ots = [], {}
                for tt in tts:
                    for nr in range(NTR):
                        def u(tt=tt, nr=nr):
                            off = (tt % 4) * P
                            if nr == 0:
                                ots[tt] = otp.tile([P, HID], F16, name="ot")
                            ot = ots[tt]
                            ns = slice(nr * 512, (nr + 1) * 512)
                            ps = psO.tile([P, 512], F32, name="psO_t")
                            for h in range(HQ):
                                nc.tensor.matmul(
                                    ps[:], ctxT[h][qr][:, off:off + P],
                                    wo_sb[:, h, ns],
                                    start=(h == 0), stop=(h == HQ - 1),
                                )
                            use_act = (oproj_n[0] % act_every
                                       == act_every - 1)
                            if use_act:
                                nc.scalar.copy(ot[:, ns], ps[:])
                            else:
                                nc.vector.tensor_copy(ot[:, ns], ps[:])
                            oproj_n[0] += 1
                            if nr == NTR - 1:
                                nc.sync.dma_start(
                                    out[tt * P:(tt + 1) * P, :], ot[:])
                        units.append(u)
                return units

            def emit_attn_a(qr, filler, extra=()):
                """Score phase for both heads of query-range qr; filler
                units (o_proj tiles of qr-1) pop every other st."""
                ex = list(extra)
                if ex:  # last-qr: qr-2's ctx blocks lead (they are the
                    # early filler and the units depend on their output)
                    m0 = [(1, ex[0]), (4, ex[1])]
                    m1 = [(2, ex[2])]
                    a0 = emit_scores(0, qr, mids=m0, filler=filler,
                                     filler_from=6)
                    a1 = emit_scores(1, qr, mids=m1, filler=filler)
                else:
                    a0 = emit_scores(0, qr, filler=filler)
                    a1 = emit_scores(1, qr, filler=filler)
                return a0, a1

            def emit_attn_b(qr, a0, a1, filler=None):
                """Context phase for both heads of query-range qr; drains
                any remaining filler units at the end."""
                emit_ctx(0, qr, *a0, filler=filler)
                emit_ctx(1, qr, *a1, filler=filler)
                while filler:
                    filler.popleft()()

            # ==================== main pipelined loop ====================
            for tr in range(NTR):
                ts = slice(tr * 512, (tr + 1) * 512)
                if tr < 2:
                    xch = xch01[tr]
                else:
                    xch = xtp.tile([P, NK, 512], F16, name="xch")
                    for kg in range(2):
                        ks = slice(kg * 8, (kg + 1) * 8)
                        nc.sync.dma_start(xch[:, ks, :], xt[:, ks, ts])
                filler = None
                if tr >= 2:
                    filler = deque(oproj_units(
                        tr - 2, list(range(4 * (tr - 2), 4 * (tr - 1)))))
                aa = emit_attn_a(tr - 1, filler) if tr >= 1 else None
                rq0 = emit_proj_mm(tr, xch, "q0")
                rq1 = emit_proj_mm(tr, xch, "q1")
                emit_bchain(tr, "q0", *rq0)
                emit_bchain(tr, "q1", *rq1)
                rk = emit_proj_mm(tr, xch, "k")
                emit_bchain(tr, "k", *rk)
                preload_exp()
                if tr < NTR - 1:
                    emit_proj_v(tr, xch)
                else:
                    xch_last = xch  # v(3) woven into attn(3)'s score phase
                if aa is not None:
                    if tr < NTR - 1:
                        emit_attn_b(tr - 1, *aa, filler=filler)
                    else:
                        # last iteration: qr=2's ctx phase is woven into
                        # attn(3)'s exp-starved score phase instead
                        aa_prev = aa
                        filler_prev = filler
            # drain any oproj(1) units not yet emitted, then the final
            # region: attn(3) with ctx(2)/v(3) as leading filler and
            # oproj(2) units woven through, then oproj(3).
            while filler_prev:
                filler_prev.popleft()()
            filler = deque(oproj_units(
                NTR - 2, list(range(4 * (NTR - 2), 4 * (NTR - 1)))))
            a2_0, a2_1 = aa_prev
            aa = emit_attn_a(
                NTR - 1, filler,
                extra=(lambda: emit_ctx(0, NTR - 2, *a2_0),
                       lambda: emit_ctx(1, NTR - 2, *a2_1),
                       lambda: emit_proj_v(NTR - 1, xch_last)),
            )
            emit_attn_b(NTR - 1, *aa, filler=filler)
            emit_oproj(NTR - 1, list(range(4 * (NTR - 1), 4 * NTR)),
                       act_every=2, split_dma=True)

    nc.compile()
    return nc


_NC_CACHE = None


def get_nc():
    global _NC_CACHE
    if _NC_CACHE is None:
        _NC_CACHE = build_nc()
    return _NC_CACHE


def make_in_maps(x, cos, sin, Wq, Wk, Wv, Wo, q_norm_w, k_norm_w):
    x = np.asarray(x, dtype=np.float32).reshape(T, HID)
    # xt: [P, NK, T] fp16, HID index = k*P + p
    xt = np.ascontiguousarray(
        x.T.reshape(NK, P, T).transpose(1, 0, 2).astype(np.float16)
    )
    cosT = np.ascontiguousarray(np.asarray(cos, np.float32).T)
    # sin, transposed, first half negated, then rolled by 64 partitions:
    # psn = rot_half-mul uses sin_sb[H:] for dst[:H] (needs -sin[:H]) and
    # sin_sb[:H] for dst[H:] (needs +sin[H:]); single add then applies RoPE.
    sin_t = np.asarray(sin, np.float32).T.copy()
    sin_t[:H] *= -1.0
    sinT = np.ascontiguousarray(np.roll(sin_t, H, axis=0))
    qwv = np.ascontiguousarray(np.asarray(q_norm_w, np.float32).reshape(D, 1))
    kwv = np.ascontiguousarray(np.asarray(k_norm_w, np.float32).reshape(D, 1))
    si = np.arange(P)[:, None, None]
    jj = np.arange(4)[None, :, None]
    qi = np.arange(512)[None, None, :]
    masks = (si + P * jj <= qi).astype(np.float16)
    Wq = np.asarray(Wq, np.float32)
    Wk = np.asarray(Wk, np.float32)
    Wv = np.asarray(Wv, np.float32)
    Wo = np.asarray(Wo, np.float32)
    in_maps = []
    for c in range(N_CORES):
        wq_c = Wq[:, c * HQ * D:(c + 1) * HQ * D]      # [HID, 256]
        wk_c = Wk[:, c * D:(c + 1) * D]                # [HID, 128]
        wv_c = Wv[:, c * D:(c + 1) * D]                # [HID, 128]
        wkv_c = np.concatenate([wk_c, wv_c], axis=1)   # [HID, 256]
        wo_c = Wo[c * HQ * D:(c + 1) * HQ * D, :]      # [256, HID]
        in_maps.append({
            "xt": xt,
            "wq0": np.ascontiguousarray(
                wq_c[:, 0:D].reshape(NK, P, D).transpose(1, 0, 2)
            ).astype(np.float16),
            "wq1": np.ascontiguousarray(
                wq_c[:, D:].reshape(NK, P, D).transpose(1, 0, 2)
            ).astype(np.float16),
            "wkv": np.ascontiguousarray(
                wkv_c.reshape(NK, P, 2 * D).transpose(1, 0, 2)
            ).astype(np.float16),
            "wo": np.ascontiguousarray(
                wo_c.reshape(HQ, P, HID).transpose(1, 0, 2)
            ).astype(np.float16),
            "cosT": cosT.astype(np.float16),
            "sinT": sinT.astype(np.float16),
            "qw": qwv,
            "kw": kwv,
            "masks": masks,
        })
    return in_maps


def kernel(x, cos, sin, Wq, Wk, Wv, Wo, q_norm_w, k_norm_w):
    nc = get_nc()
    in_maps = make_in_maps(x, cos, sin, Wq, Wk, Wv, Wo, q_norm_w, k_norm_w)
    res = run_bass_kernel_spmd(nc, in_maps, core_ids=list(range(N_CORES)))
    acc = np.zeros((T, HID), dtype=np.float32)
    for c in range(N_CORES):
        acc += res.results[c]["out"]
    return acc.reshape(1, T, HID)


# revision 111
# speedup vs baseline: 1.0159x; 1.0024x over previous
"""GQA attention block (B=1, T=2048, HID=2048, NQ=16, NKV=8, D=128) on 8 TRN2
NeuronCores.

Sharding: tensor-parallel over heads. Core c owns q-heads {2c, 2c+1} and
kv-head c. Each core computes, from the full x:
  Q^T/K^T shards (d on partitions) and V in [token, d] layout (via a
  transposed projection, so no PE transposes)  ->  per-head RMSNorm + RoPE
  ->  causal softmax attention (no max-subtraction; scores are O(5) for
  RMS-normed q/k; diagonal blocks trimmed to the causal width)  ->  partial
  o_proj with Wo row-shard, written out fp16.
The 8 partial [T, HID] outputs are summed on the host (the row-parallel
"unshard" step).

All matmul operands are fp16 (full PE rate at any free-dim size; rel err
~5e-4, far under the 2e-2 gate -- fp8 would blow the budget); PSUM
accumulation stays fp32. Softmax denominators are accumulated on the vector
engine (fp16, 2x mode) and partition-all-reduced on the otherwise-idle Pool
engine, keeping the PE free of reduction matmuls; RMS statistics use the
same trick. Scheduling is a single tile scope, software-pipelined so no
phase barrier drains the machine:
  - iteration tr: attention score phase for qr=tr-1 (its kT/qT completed
    mid-previous-iteration; the q projections behind it absorb the exp
    drain) -> q projections -> deferred RMS/RoPE chains (their Act Sqrt
    would otherwise head-of-line block the exps; a dummy exp then
    prefetches the Act table flip) -> k/v projections -> attention context
    phase,
  - o_proj is emitted as single-PSUM-tile units woven into the exp-starved
    score stretches (Act exp throughput is the binding resource there),
    with PSUM->SBUF fp16 copies split between DVE and Act,
  - softmax normalization runs per 256-query half so o_proj units' ctxT
    dependencies resolve early; the last query-range's attention weaves
    qr=2's context phase and v(3) in as extra PE filler.
DMA: transfers are FIFO in desc-gen order, so startup-critical loads ride
the Act HWDGE queue in exact need-order while x0 drips on SP; out-writes
are batched per 128-row block (HWDGE desc-gen is a serial 625ns/op
resource).
"""

import sys
from collections import deque

sys.path.insert(0, "/opt/trn_rl_repo")

import numpy as np

import concourse.bass as bass  # noqa: F401  (bass must import before tile)
import concourse.bass_isa as bass_isa
import concourse.mybir as mybir
import concourse.tile as tile
from concourse import bacc
from concourse.bass_utils import run_bass_kernel_spmd

N_CORES = 8
T = 2048
HID = 2048
NQ, NKV, D = 16, 8, 128
HQ = NQ // N_CORES  # q heads per core = 2
EPS = 1e-6
SCALE = D**-0.5

P = 128
H = D // 2           # rope half
NK = HID // P        # 16 k-chunks for projections
NTR = T // 512       # 4 T-ranges of 512
NTT = T // P         # 16 T-tiles of 128

F32 = mybir.dt.float32
F16 = mybir.dt.float16
ACT_EXP = mybir.ActivationFunctionType.Exp
ACT_SQUARE = mybir.ActivationFunctionType.Square
ACT_SQRT = mybir.ActivationFunctionType.Sqrt
RED_ADD = bass_isa.ReduceOp.add


def build_nc():
    nc = bacc.Bacc("TRN2", target_bir_lowering=False, debug=False,
                   num_devices=N_CORES)

    # ---- DRAM tensors (names = in_map keys); all pre-arranged on host ----
    xt = nc.dram_tensor("xt", [P, NK, T], F16, kind="ExternalInput")
    wq0 = nc.dram_tensor("wq0", [P, NK, D], F16, kind="ExternalInput")
    wq1 = nc.dram_tensor("wq1", [P, NK, D], F16, kind="ExternalInput")
    wkv = nc.dram_tensor("wkv", [P, NK, 2 * D], F16, kind="ExternalInput")
    wo = nc.dram_tensor("wo", [P, HQ, HID], F16, kind="ExternalInput")
    cosT = nc.dram_tensor("cosT", [D, T], F16, kind="ExternalInput")
    sinT = nc.dram_tensor("sinT", [D, T], F16, kind="ExternalInput")
    qw = nc.dram_tensor("qw", [D, 1], F32, kind="ExternalInput")
    kw = nc.dram_tensor("kw", [D, 1], F32, kind="ExternalInput")
    masks = nc.dram_tensor("masks", [P, 4, 512], F16, kind="ExternalInput")
    out = nc.dram_tensor("out", [T, HID], F16, kind="ExternalOutput")

    with tile.TileContext(nc) as tc:
        with (
            tc.tile_pool(name="cst", bufs=1) as cst,
            tc.tile_pool(name="fin", bufs=1) as fin,
            tc.tile_pool(name="xtp", bufs=2) as xtp,
            tc.tile_pool(name="rawp", bufs=2) as rawp,
            tc.tile_pool(name="sqp", bufs=3) as sqp,
            tc.tile_pool(name="rmsp", bufs=2) as rmsp,
            tc.tile_pool(name="ropep", bufs=3) as ropep,
            tc.tile_pool(name="atp", bufs=36) as atp,
            tc.tile_pool(name="accp", bufs=4) as accp,
            tc.tile_pool(name="dnp", bufs=3) as dnp,
            tc.tile_pool(name="otp", bufs=4) as otp,
            tc.tile_pool(name="psA", bufs=2, space="PSUM") as psA,
            tc.tile_pool(name="psS", bufs=3, space="PSUM") as psS,
            tc.tile_pool(name="psO", bufs=3, space="PSUM") as psO,
        ):
            # ---------- constants / weights resident in SBUF ----------
            wq_sb = [cst.tile([P, NK, D], F16, name=f"wq{h}_sb")
                     for h in range(HQ)]
            wkv_sb = cst.tile([P, NK, 2 * D], F16)
            wo_sb = cst.tile([P, HQ, HID], F16)
            cos_sb = cst.tile([P, T], F16)
            sin_sb = cst.tile([P, T], F16)   # pre-rolled, first half negated
            masks_sb = cst.tile([P, 4, 512], F16)
            qw_sb = cst.tile([P, 1], F32)
            kw_sb = cst.tile([P, 1], F32)
            eps_sb = cst.tile([P, 1], F32)
            nc.gpsimd.memset(eps_sb[:], EPS)
            dmy = cst.tile([1, 1], F32)
            nc.gpsimd.memset(dmy[:], 0.0)
            dmy_o = cst.tile([1, 1], F16)

            def preload_exp():
                # A do-nothing exp: forces the Act function-table switch
                # (1.3us) to happen NOW, while Act is idle, instead of right
                # in front of the first real exp of the attention phase.
                nc.scalar.activation(dmy_o[:], dmy[:], ACT_EXP)

            # DMA transfers run FIFO in desc-gen order, and desc-gen follows
            # per-queue program order -- so everything startup-critical rides
            # the Act HWDGE queue in exact need-order (weights, then x1 and
            # trig tables interleaved), x0/x2/x3/out ride SP, and wo (needed
            # only by o_proj at ~40us) is emitted inside iteration 1 on the
            # Pool SWDGE queue so it cannot jump the early queue.
            # x0's first drip goes before the weights so its desc-gen (and
            # transfer) is first in the FIFO; the rest interleave by need.
            xch01 = []
            for tr01 in range(2):
                xch_t = xtp.tile([P, NK, 512], F16, name="xch")
                t0 = slice(tr01 * 512, (tr01 + 1) * 512)
                if tr01 == 0:
                    # drip x0 in 4 chunks so the first projection group can
                    # chew k-chunks while the rest stream in
                    for ks in (slice(0, 2), slice(2, 5), slice(5, 10),
                               slice(10, 16)):
                        nc.sync.dma_start(xch_t[:, ks, :], xt[:, ks, t0])
                    nc.scalar.dma_start(wq_sb[0][:], wq0[:])
                    nc.scalar.dma_start(qw_sb[:], qw[:])
                    nc.scalar.dma_start(kw_sb[:], kw[:])
                    nc.scalar.dma_start(wq_sb[1][:], wq1[:])
                    nc.scalar.dma_start(wkv_sb[:], wkv[:])
                else:
                    nc.scalar.dma_start(xch_t[:, 0:6, :], xt[:, 0:6, t0])
                    nc.scalar.dma_start(xch_t[:, 6:11, :],
                                        xt[:, 6:11, t0])
                    nc.scalar.dma_start(xch_t[:, 11:, :], xt[:, 11:, t0])
                    nc.scalar.dma_start(cos_sb[:], cosT[:])
                    nc.scalar.dma_start(sin_sb[:], sinT[:])
                    nc.scalar.dma_start(masks_sb[:], masks[:])
                    nc.scalar.dma_start(wo_sb[:], wo[:])
                xch01.append(xch_t)

            # final (post RMS+RoPE) activations, fp16
            qT = [fin.tile([P, T], F16, name=f"qT{h}") for h in range(HQ)]
            kT = fin.tile([P, T], F16)
            vnat = fin.tile([P, NTT, D], F16)
            ctxT = [
                [fin.tile([P, 512], F16, name=f"ctxT{h}_{qr}")
                 for qr in range(NTR)]
                for h in range(HQ)
            ]

            COLS = {
                "q0": (qT[0], (lambda: wq_sb[0]), 0, qw_sb),
                "q1": (qT[1], (lambda: wq_sb[1]), 0, qw_sb),
                "k": (kT, (lambda: wkv_sb), 0, kw_sb),
            }

            def emit_proj_mm(tr, xch, name):
                """Projection matmul group + psum evacuation (Act Copy +
                Square only -- both table-neutral, so they never delay the
                attention exps that follow on the Act queue)."""
                _, w_fn, off, _ = COLS[name]
                w_sb = w_fn()
                ps = psA.tile([P, 512], F32, name="psA_t")
                for k in range(NK):
                    nc.tensor.matmul(
                        ps[:], w_sb[:, k, off:off + D], xch[:, k, :],
                        start=(k == 0), stop=(k == NK - 1),
                    )
                sq = sqp.tile([P, 512], F16, name="sq")
                nc.scalar.activation(sq[:], ps[:], ACT_SQUARE)
                raw = rawp.tile([P, 512], F32, name=f"raw_{name}")
                nc.scalar.copy(raw[:], ps[:])
                return raw, sq

            def emit_bchain(tr, name, raw, sq):
                """Deferred RMS+RoPE chain (contains the Act Sqrt, so it is
                emitted AFTER the attention score phase: its table flip then
                lands in Act slack, not in front of the exps)."""
                ts = slice(tr * 512, (tr + 1) * 512)
                dst, _, _, nrm_w = COLS[name]
                ssum_b = rmsp.tile([P, 512], F32, name="ssum_b")
                nc.gpsimd.partition_all_reduce(ssum_b[:], sq[:],
                                               channels=P,
                                               reduce_op=RED_ADD)
                rstd_b = rmsp.tile([P, 512], F32, name="rstd_b")
                nc.scalar.activation(rstd_b[:], ssum_b[:], ACT_SQRT,
                                     scale=1.0 / D, bias=eps_sb[:])
                rinv_b = rmsp.tile([P, 512], F32, name="rinv_b")
                nc.vector.reciprocal_approx_fast(rinv_b[:], rstd_b[:])
                nq = ropep.tile([P, 512], F16, name="nq")
                nc.vector.scalar_tensor_tensor(
                    nq[:], raw[:], nrm_w[:], rinv_b[:],
                    mybir.AluOpType.mult, mybir.AluOpType.mult,
                )
                pc = ropep.tile([P, 512], F16, name="pc")
                nc.vector.tensor_mul(pc[:], nq[:], cos_sb[:, ts])
                psn = ropep.tile([P, 512], F16, name="psn")
                nc.vector.tensor_mul(psn[0:H, :], nq[H:D, :],
                                     sin_sb[H:D, ts])
                nc.vector.tensor_mul(psn[H:D, :], nq[0:H, :],
                                     sin_sb[0:H, ts])
                nc.vector.tensor_add(dst[:, ts], pc[:], psn[:])

            def emit_proj_v(tr, xch):
                # --- v : out [128 t, 128 d] per t-tile, no transpose
                vps = psA.tile([P, 512], F32, name="psA_t")
                for j in range(4):
                    for k in range(NK):
                        nc.tensor.matmul(
                            vps[:, j * P:(j + 1) * P],
                            xch[:, k, j * P:(j + 1) * P],
                            wkv_sb[:, k, D:2 * D],
                            start=(k == 0), stop=(k == NK - 1),
                        )
                nc.vector.tensor_copy(vnat[:, 4 * tr:4 * tr + 4, :],
                                      vps[:])

            def emit_scores(h, qr, mids=(), filler=None, filler_from=0):
                """Score matmuls + exp + causal mask + fp16 denominator
                accumulation for one (head, query-range). Returns (ats,
                qoffs, acc). mids: (st, callback) pairs weaving in big PE
                blocks; filler: a deque of small PE units (single o_proj
                tiles) popped every other st to soak up exp latency."""
                n_st = 4 * (qr + 1)
                ats, qoffs = [], []
                acc = accp.tile([P, 512], F16, name="acc")
                for st in range(n_st):
                    for st_m, cb in mids:
                        if st == st_m:
                            cb()
                    if filler and st % 2 == 1 and st >= filler_from:
                        filler.popleft()()
                    j = st - 4 * qr
                    qoff = max(0, 128 * j) if j >= 0 else 0
                    s_ps = psS.tile([P, 512], F32, name="s_ps")
                    nc.tensor.matmul(
                        s_ps[:, qoff:], kT[:, st * P:(st + 1) * P],
                        qT[h][:, qr * 512 + qoff:(qr + 1) * 512],
                        start=True, stop=True,
                    )
                    at = atp.tile([P, 512], F16, name="at")
                    nc.scalar.activation(at[:, qoff:], s_ps[:, qoff:],
                                         ACT_EXP, scale=SCALE)
                    if j >= 0:
                        nc.vector.tensor_mul(at[:, qoff:], at[:, qoff:],
                                             masks_sb[:, j, qoff:])
                    # accumulate into acc on DVE (fp16 2x mode)
                    if st == 1:
                        if qoffs[0] == 0 and qoff == 0:
                            nc.vector.tensor_add(acc[:], ats[0][:], at[:])
                        else:  # qr == 0: at0 full, at1 starts at 128
                            nc.vector.tensor_copy(acc[:, :qoff],
                                                  ats[0][:, :qoff])
                            nc.vector.tensor_add(acc[:, qoff:],
                                                 ats[0][:, qoff:],
                                                 at[:, qoff:])
                    elif st >= 2:
                        nc.vector.tensor_add(acc[:, qoff:], acc[:, qoff:],
                                             at[:, qoff:])
                    ats.append(at)
                    qoffs.append(qoff)
                return ats, qoffs, acc

            def emit_ctx(h, qr, ats, qoffs, acc, filler=None):
                """Context matmuls + denominator all-reduce + normalize.
                For the final query-range the normalize runs per 128-query
                quarter so o_proj(3) t-tiles can start progressively."""
                n_st = 4 * (qr + 1)
                ctx_ps = psO.tile([P, 512], F32, name="psO_t")
                for st in range(n_st):
                    if filler and st % 2 == 1:
                        filler.popleft()()
                    qoff = qoffs[st]
                    nc.tensor.matmul(
                        ctx_ps[:, qoff:], vnat[:, st, :], ats[st][:, qoff:],
                        start=(st == 0), stop=(st == n_st - 1),
                    )
                denom_b = dnp.tile([P, 512], F32, name="denom_b")
                rb = dnp.tile([P, 512], F32, name="rb")
                nc.gpsimd.partition_all_reduce(denom_b[:], acc[:],
                                               channels=P,
                                               reduce_op=RED_ADD)
                for qs4 in (slice(0, 256), slice(256, 512)):
                    nc.vector.reciprocal_approx_fast(rb[:, qs4],
                                                     denom_b[:, qs4])
                    nc.vector.tensor_mul(ctxT[h][qr][:, qs4],
                                         ctx_ps[:, qs4], rb[:, qs4])

            oproj_n = [0]

            def emit_oproj(qr, tts, act_every=4, split_dma=False):
                """o_proj for t-tiles tts of query-range qr. PSUM -> fp16
                SBUF copies (GPSIMD can't read PSUM on hw) split between DVE
                and Act (every act_every-th on Act); the 4 n-range tiles of
                one t-tile stage into one [P, HID] buffer so a single
                batched DMA covers the whole row block (HWDGE desc-gen is a
                serial 625ns/op resource -- 16 DMAs, not 64). split_dma
                switches the last tile back to per-n-range DMAs so the
                drain tail is fine-grained."""
                for tt in tts:
                    off = (tt % 4) * P
                    ot = otp.tile([P, HID], F16, name="ot")
                    last = split_dma and tt == tts[-1]
                    for nr in range(NTR):
                        ns = slice(nr * 512, (nr + 1) * 512)
                        ps = psO.tile([P, 512], F32, name="psO_t")
                        for h in range(HQ):
                            nc.tensor.matmul(
                                ps[:], ctxT[h][qr][:, off:off + P],
                                wo_sb[:, h, ns],
                                start=(h == 0), stop=(h == HQ - 1),
                            )
                        use_act = oproj_n[0] % act_every == act_every - 1
                        if use_act:
                            nc.scalar.copy(ot[:, ns], ps[:])
                        else:
                            nc.vector.tensor_copy(ot[:, ns], ps[:])
                        oproj_n[0] += 1
                        if last:
                            nc.sync.dma_start(
                                out[tt * P:(tt + 1) * P, ns], ot[:, ns])
                    if not last:
                        nc.sync.dma_start(out[tt * P:(tt + 1) * P, :], ot[:])

            def oproj_units(qr, tts, act_every=4):
                """Per-(t-tile, n-range) o_proj emission units for fine
                weaving into exp-starved stretches."""
                units, ots = [], {}
                for tt in tts:
                    for nr in range(NTR):
                        def u(tt=tt, nr=nr):
                            off = (tt % 4) * P
                            if nr == 0:
                                ots[tt] = otp.tile([P, HID], F16, name="ot")
                            ot = ots[tt]
                            ns = slice(nr * 512, (nr + 1) * 512)
                            ps = psO.tile([P, 512], F32, name="psO_t")
                            for h in range(HQ):
                                nc.tensor.matmul(
                                    ps[:], ctxT[h][qr][:, off:off + P],
                                    wo_sb[:, h, ns],
                                    start=(h == 0), stop=(h == HQ - 1),
                                )
                            use_act = (oproj_n[0] % act_every
                                       == act_every - 1)
                            if use_act:
                                nc.scalar.copy(ot[:, ns], ps[:])
                            else:
                                nc.vector.tensor_copy(ot[:, ns], ps[:])
                            oproj_n[0] += 1
                            if nr == NTR - 1:
                                nc.sync.dma_start(
                                    out[tt * P:(tt + 1) * P, :], ot[:])
                        units.append(u)
                return units

            def emit_attn_a(qr, filler, extra=()):
                """Score phase for both heads of query-range qr; filler
                units (o_proj tiles of qr-1) pop every other st."""
                ex = list(extra)
                if ex:  # last-qr: qr-2's ctx blocks lead (they are the
                    # early filler and the units depend on their output)
                    m0 = [(1, ex[0]), (4, ex[1])]
                    m1 = [(2, ex[2])]
                    a0 = emit_scores(0, qr, mids=m0, filler=filler,
                                     filler_from=6)
                    a1 = emit_scores(1, qr, mids=m1, filler=filler)
                else:
                    a0 = emit_scores(0, qr, filler=filler)
                    a1 = emit_scores(1, qr, filler=filler)
                return a0, a1

            def emit_attn_b(qr, a0, a1, filler=None):
                """Context phase for both heads of query-range qr; drains
                any remaining filler units at the end."""
                emit_ctx(0, qr, *a0, filler=filler)
                emit_ctx(1, qr, *a1, filler=filler)
                while filler:
                    filler.popleft()()

            # ==================== main pipelined loop ====================
            for tr in range(NTR):
                ts = slice(tr * 512, (tr + 1) * 512)
                if tr < 2:
                    xch = xch01[tr]
                else:
                    xch = xtp.tile([P, NK, 512], F16, name="xch")
                    for kg in range(2):
                        ks = slice(kg * 8, (kg + 1) * 8)
                        nc.sync.dma_start(xch[:, ks, :], xt[:, ks, ts])
                filler = None
                if tr >= 2:
                    filler = deque(oproj_units(
                        tr - 2, list(range(4 * (tr - 2), 4 * (tr - 1)))))
                aa = emit_attn_a(tr - 1, filler) if tr >= 1 else None
                rq0 = emit_proj_mm(tr, xch, "q0")
                rq1 = emit_proj_mm(tr, xch, "q1")
                emit_bchain(tr, "q0", *rq0)
                emit_bchain(tr, "q1", *rq1)
                rk = emit_proj_mm(tr, xch, "k")
                emit_bchain(tr, "k", *rk)
                preload_exp()
                if tr < NTR - 1:
                    emit_proj_v(tr, xch)
                else:
                    xch_last = xch  # v(3) woven into attn(3)'s score phase
                if aa is not None:
                    if tr < NTR - 1:
                        emit_attn_b(tr - 1, *aa, filler=filler)
                    else:
                        # last iteration: qr=2's ctx phase is woven into
                        # attn(3)'s exp-starved score phase instead
                        aa_prev = aa
                        filler_prev = filler
            # drain any oproj(1) units not yet emitted, then the final
            # region: attn(3) with ctx(2)/v(3) as leading filler and
            # oproj(2) units woven through, then oproj(3).
            while filler_prev:
                filler_prev.popleft()()
            filler = deque(oproj_units(
                NTR - 2, list(range(4 * (NTR - 2), 4 * (NTR - 1)))))
            a2_0, a2_1 = aa_prev
            aa = emit_attn_a(
                NTR - 1, filler,
                extra=(lambda: emit_ctx(0, NTR - 2, *a2_0),
                       lambda: emit_ctx(1, NTR - 2, *a2_1),
                       lambda: emit_proj_v(NTR - 1, xch_last)),
            )
            emit_attn_b(NTR - 1, *aa, filler=filler)
            emit_oproj(NTR - 1, list(range(4 * (NTR - 1), 4 * NTR)),
                       act_every=2, split_dma=True)

    nc.compile()
    return nc


_NC_CACHE = None


def get_nc():
    global _NC_CACHE
    if _NC_CACHE is None:
        _NC_CACHE = build_nc()
    return _NC_CACHE


def make_in_maps(x, cos, sin, Wq, Wk, Wv, Wo, q_norm_w, k_norm_w):
    x = np.asarray(x, dtype=np.float32).reshape(T, HID)
    # xt: [P, NK, T] fp16, HID index = k*P + p
    xt = np.ascontiguousarray(
        x.T.reshape(NK, P, T).transpose(1, 0, 2).astype(np.float16)
    )
    cosT = np.ascontiguousarray(np.asarray(cos, np.float32).T)
    # sin, transposed, first half negated, then rolled by 64 partitions:
    # psn = rot_half-mul uses sin_sb[H:] for dst[:H] (needs -sin[:H]) and
    # sin_sb[:H] for dst[H:] (needs +sin[H:]); single add then applies RoPE.
    sin_t = np.asarray(sin, np.float32).T.copy()
    sin_t[:H] *= -1.0
    sinT = np.ascontiguousarray(np.roll(sin_t, H, axis=0))
    qwv = np.ascontiguousarray(np.asarray(q_norm_w, np.float32).reshape(D, 1))
    kwv = np.ascontiguousarray(np.asarray(k_norm_w, np.float32).reshape(D, 1))
    si = np.arange(P)[:, None, None]
    jj = np.arange(4)[None, :, None]
    qi = np.arange(512)[None, None, :]
    masks = (si + P * jj <= qi).astype(np.float16)
    Wq = np.asarray(Wq, np.float32)
    Wk = np.asarray(Wk, np.float32)
    Wv = np.asarray(Wv, np.float32)
    Wo = np.asarray(Wo, np.float32)
    in_maps = []
    for c in range(N_CORES):
        wq_c = Wq[:, c * HQ * D:(c + 1) * HQ * D]      # [HID, 256]
        wk_c = Wk[:, c * D:(c + 1) * D]                # [HID, 128]
        wv_c = Wv[:, c * D:(c + 1) * D]                # [HID, 128]
        wkv_c = np.concatenate([wk_c, wv_c], axis=1)   # [HID, 256]
        wo_c = Wo[c * HQ * D:(c + 1) * HQ * D, :]      # [256, HID]
        in_maps.append({
            "xt": xt,
            "wq0": np.ascontiguousarray(
                wq_c[:, 0:D].reshape(NK, P, D).transpose(1, 0, 2)
            ).astype(np.float16),
            "wq1": np.ascontiguousarray(
                wq_c[:, D:].reshape(NK, P, D).transpose(1, 0, 2)
            ).astype(np.float16),
            "wkv": np.ascontiguousarray(
                wkv_c.reshape(NK, P, 2 * D).transpose(1, 0, 2)
            ).astype(np.float16),
            "wo": np.ascontiguousarray(
                wo_c.reshape(HQ, P, HID).transpose(1, 0, 2)
            ).astype(np.float16),
            "cosT": cosT.astype(np.float16),
            "sinT": sinT.astype(np.float16),
            "qw": qwv,
            "kw": kwv,
            "masks": masks,
        })
    return in_maps


def kernel(x, cos, sin, Wq, Wk, Wv, Wo, q_norm_w, k_norm_w):
    nc = get_nc()
    in_maps = make_in_maps(x, cos, sin, Wq, Wk, Wv, Wo, q_norm_w, k_norm_w)
    res = run_bass_kernel_spmd(nc, in_maps, core_ids=list(range(N_CORES)))
    acc = np.zeros((T, HID), dtype=np.float32)
    for c in range(N_CORES):
        acc += res.results[c]["out"]
    return acc.reshape(1, T, HID)
